# revision 1
# baseline (speedup 1.0000x reference)
"""GRUCell fused kernel for Trainium2, data-parallel over 8 NeuronCores.

Strategy:
  - Shard batch (16384) across 8 cores -> 2048 rows/core; replicate weights.
  - Host-side: feed activations feature-major (x.T, h.T per shard) and
    weights packed per output j-tile in exact consumption order, so the
    device never transposes anything and the PE pipeline starts after
    ~1.5MB of DMA instead of the full 6MB weight set.
  - Device: out.T tiles [128 h-units, 512 batch] computed as
    W.T-slices (stationary) x act.T (moving) matmuls in float32r
    (1 cycle/row at N=512; true fp32 is 4x slower), fp32 PSUM accumulate.
    Gate order ig -> hg -> r -> z so early gates only need small weights.
    Epilogue on ACT (sigmoid/tanh with fused bias) + DVE
    (scalar_tensor_tensor to fold remaining biases).
  - h' = n + z*(h - n) where n = tanh(i_g + r*h_g).
"""

import os
import numpy as np
from contextlib import ExitStack

import concourse.bass as bass
import concourse.tile as tile
from concourse import bacc, mybir
from concourse.bass_utils import run_bass_kernel_spmd

B, I, H = 16384, 512, 512
NCORES = 8
BL = B // NCORES          # 2048 rows per core
NB = 512                  # batch tile (matmul moving free dim)
NBT = BL // NB            # 4 batch tiles per core
P = 128                   # partitions
KX = I // P               # 4 k-tiles over input features
KH = H // P               # 4 k-tiles over hidden features
JT = H // P               # 4 output j-tiles per gate

FP32 = mybir.dt.float32
FP32R = mybir.dt.float32r

_cache = {}


def build_gru_bass():
    """Build (once) the SPMD Bass program for one core's shard."""
    if "nc" in _cache:
        return _cache["nc"]

    nc = bacc.Bacc(
        "TRN2",
        target_bir_lowering=False,
        debug=False,
        enable_asserts=False,
        num_devices=NCORES,
    )

    xT = nc.dram_tensor("xT", [I, BL], FP32R, kind="ExternalInput").ap()
    hT = nc.dram_tensor("hT", [H, BL], FP32R, kind="ExternalInput").ap()
    # packed weights per j-tile: [JT, 128, 3072]; free-dim column groups:
    #   [0:512)      w_i blocks kt=0..3   (W_i.T  [kt*128:+128, jt*128:+128])
    #   [512:1024)   w_h blocks kt=0..3
    #   [1024:2048)  w_r blocks kt=0..7   (W_gate.T cols jt*128:+128)
    #   [2048:3072)  w_z blocks kt=0..7   (W_gate.T cols 512+jt*128:+128)
    wpk = nc.dram_tensor("wpk", [JT, P, 3072], FP32R, kind="ExternalInput").ap()
    # bias columns: 0..3 b_r per j-tile, 4..7 b_z, 8..11 b_i, 12..15 b_h
    bias = nc.dram_tensor("bias", [P, 16], FP32, kind="ExternalInput").ap()
    outT = nc.dram_tensor("outT", [H, BL], FP32, kind="ExternalOutput").ap()

    ADD = mybir.AluOpType.add
    MULT = mybir.AluOpType.mult
    SIG = mybir.ActivationFunctionType.Sigmoid
    TANH = mybir.ActivationFunctionType.Tanh

    with tile.TileContext(nc) as tc, ExitStack() as ctx:
        wpool = ctx.enter_context(tc.tile_pool(name="weights", bufs=1))
        apool = ctx.enter_context(tc.tile_pool(name="acts", bufs=2))
        ppool = ctx.enter_context(tc.tile_pool(name="psum", bufs=2, space="PSUM"))
        epool = ctx.enter_context(tc.tile_pool(name="epi", bufs=3))

        bias_s = wpool.tile([P, 16], FP32, tag="bias", name="bias_s")
        nc.sync.dma_start(out=bias_s[:], in_=bias[:, :])

        # bt0 activations first: the first matmuls only need xt + w_i of jt0.
        xt_all = [[None] * KX for _ in range(NBT)]
        ht_all = [[None] * KH for _ in range(NBT)]

        def load_acts(bt):
            bsl = bass.ts(bt, NB)
            for kt in range(KX):
                xtile = apool.tile([P, NB], FP32R, tag=f"xt{kt}",
                                   name=f"xt{kt}_{bt}")
                nc.sync.dma_start(out=xtile[:], in_=xT[kt * P:(kt + 1) * P, bsl])
                xt_all[bt][kt] = xtile
            for kt in range(KH):
                htile = apool.tile([P, NB], FP32R, tag=f"ht{kt}",
                                   name=f"ht{kt}_{bt}")
                nc.sync.dma_start(out=htile[:], in_=hT[kt * P:(kt + 1) * P, bsl])
                ht_all[bt][kt] = htile

        # Interleave the first batch-tile's activation loads with jt0's
        # weight groups in exact first-use order, so the first matmul's
        # inputs complete after ~0.8MB of DMA instead of ~2.5MB.
        w_i, w_h, w_r, w_z = [None] * JT, [None] * JT, [None] * JT, [None] * JT

        def load_wgroup(jt, which):
            col0 = {"i": 0, "h": 512, "r": 1024, "z": 2048}[which]
            width = 512 if which in ("i", "h") else 1024
            wt = wpool.tile([P, width], FP32R, tag=f"w{which}{jt}",
                            name=f"w{which}{jt}")
            nc.sync.dma_start(out=wt[:], in_=wpk[jt, :, col0:col0 + width])
            {"i": w_i, "h": w_h, "r": w_r, "z": w_z}[which][jt] = wt

        bsl0 = bass.ts(0, NB)
        for kt in range(KX):
            xtile = apool.tile([P, NB], FP32R, tag=f"xt{kt}", name=f"xt{kt}_0")
            nc.sync.dma_start(out=xtile[:], in_=xT[kt * P:(kt + 1) * P, bsl0])
            xt_all[0][kt] = xtile
        load_wgroup(0, "i")
        for kt in range(KH):
            htile = apool.tile([P, NB], FP32R, tag=f"ht{kt}", name=f"ht{kt}_0")
            nc.sync.dma_start(out=htile[:], in_=hT[kt * P:(kt + 1) * P, bsl0])
            ht_all[0][kt] = htile
        load_wgroup(0, "h")
        load_wgroup(0, "r")
        load_wgroup(0, "z")
        for jt in range(1, JT):
            for which in ("i", "h", "r", "z"):
                load_wgroup(jt, which)

        # ---- main loop over batch tiles ----
        for bt in range(NBT):
            bsl = bass.ts(bt, NB)
            if bt > 0:
                load_acts(bt)
            xt = xt_all[bt]
            ht = ht_all[bt]

            for jt in range(JT):
                j0 = jt * P
                # i_gate pre-activation: K = I
                ig_ps = ppool.tile([P, NB], FP32, tag="ig_ps", name=f"ig_ps_{bt}_{jt}")
                for kt in range(KX):
                    nc.tensor.matmul(
                        out=ig_ps[:], lhsT=w_i[jt][:, kt * P:(kt + 1) * P],
                        rhs=xt[kt][:], start=(kt == 0), stop=(kt == KX - 1))
                # h_gate pre-activation: K = H
                hg_ps = ppool.tile([P, NB], FP32, tag="hg_ps", name=f"hg_ps_{bt}_{jt}")
                for kt in range(KH):
                    nc.tensor.matmul(
                        out=hg_ps[:], lhsT=w_h[jt][:, kt * P:(kt + 1) * P],
                        rhs=ht[kt][:], start=(kt == 0), stop=(kt == KH - 1))
                # r gate pre-activation: K = I + H
                r_ps = ppool.tile([P, NB], FP32, tag="r_ps", name=f"r_ps_{bt}_{jt}")
                for kt in range(KX):
                    nc.tensor.matmul(
                        out=r_ps[:], lhsT=w_r[jt][:, kt * P:(kt + 1) * P],
                        rhs=xt[kt][:], start=(kt == 0), stop=False)
                for kt in range(KH):
                    nc.tensor.matmul(
                        out=r_ps[:], lhsT=w_r[jt][:, (KX + kt) * P:(KX + kt + 1) * P],
                        rhs=ht[kt][:], start=False, stop=(kt == KH - 1))
                # z gate pre-activation: K = I + H
                z_ps = ppool.tile([P, NB], FP32, tag="z_ps", name=f"z_ps_{bt}_{jt}")
                for kt in range(KX):
                    nc.tensor.matmul(
                        out=z_ps[:], lhsT=w_z[jt][:, kt * P:(kt + 1) * P],
                        rhs=xt[kt][:], start=(kt == 0), stop=False)
                for kt in range(KH):
                    nc.tensor.matmul(
                        out=z_ps[:], lhsT=w_z[jt][:, (KX + kt) * P:(KX + kt + 1) * P],
                        rhs=ht[kt][:], start=False, stop=(kt == KH - 1))

                # ---- epilogue ----
                r_s = epool.tile([P, NB], FP32, tag="r_s", name=f"r_s_{bt}_{jt}")
                nc.scalar.activation(out=r_s[:], in_=r_ps[:], func=SIG,
                                     bias=bias_s[:, jt:jt + 1])
                z_s = epool.tile([P, NB], FP32, tag="z_s", name=f"z_s_{bt}_{jt}")
                nc.scalar.activation(out=z_s[:], in_=z_ps[:], func=SIG,
                                     bias=bias_s[:, 4 + jt:5 + jt])
                # m = (h_gate + b_h) * r
                m = epool.tile([P, NB], FP32, tag="m", name=f"m_{bt}_{jt}")
                nc.vector.scalar_tensor_tensor(
                    out=m[:], in0=hg_ps[:], scalar=bias_s[:, 12 + jt:13 + jt],
                    in1=r_s[:], op0=ADD, op1=MULT)
                # s = (i_gate + b_i) + m
                s = epool.tile([P, NB], FP32, tag="s", name=f"s_{bt}_{jt}")
                nc.vector.scalar_tensor_tensor(
                    out=s[:], in0=ig_ps[:], scalar=bias_s[:, 8 + jt:9 + jt],
                    in1=m[:], op0=ADD, op1=ADD)
                n = epool.tile([P, NB], FP32, tag="n", name=f"n_{bt}_{jt}")
                nc.scalar.activation(out=n[:], in_=s[:], func=TANH)
                # out = n + z * (h - n)
                d = epool.tile([P, NB], FP32, tag="d", name=f"d_{bt}_{jt}")
                nc.vector.tensor_sub(d[:], ht[jt][:].bitcast(FP32), n[:])
                e = epool.tile([P, NB], FP32, tag="e", name=f"e_{bt}_{jt}")
                nc.vector.tensor_mul(e[:], z_s[:], d[:])
                o = epool.tile([P, NB], FP32, tag="o", name=f"o_{bt}_{jt}")
                nc.vector.tensor_add(o[:], n[:], e[:])
                nc.sync.dma_start(out=outT[j0:j0 + P, bsl], in_=o[:])

    nc.compile()
    _cache["nc"] = nc
    return nc


def kernel(input, hidden, W_gate, b_gate, W_i, b_i, W_h, b_h):
    input = np.asarray(input, dtype=np.float32)
    hidden = np.asarray(hidden, dtype=np.float32)
    W_gate = np.asarray(W_gate, dtype=np.float32)
    b_gate = np.asarray(b_gate, dtype=np.float32)
    W_i = np.asarray(W_i, dtype=np.float32)
    b_i = np.asarray(b_i, dtype=np.float32)
    W_h = np.asarray(W_h, dtype=np.float32)
    b_h = np.asarray(b_h, dtype=np.float32)

    nc = build_gru_bass()

    wgT = W_gate.T            # [I+H, 2H]
    wiT = W_i.T               # [I, H]
    whT = W_h.T               # [H, H]
    wpk = np.empty((JT, P, 3072), dtype=np.float32)
    for jt in range(JT):
        jsl = slice(jt * P, (jt + 1) * P)
        for kt in range(KX):
            wpk[jt, :, kt * P:(kt + 1) * P] = wiT[kt * P:(kt + 1) * P, jsl]
        for kt in range(KH):
            wpk[jt, :, 512 + kt * P:512 + (kt + 1) * P] = \
                whT[kt * P:(kt + 1) * P, jsl]
        for kt in range(KX + KH):
            wpk[jt, :, 1024 + kt * P:1024 + (kt + 1) * P] = \
                wgT[kt * P:(kt + 1) * P, jsl]
            wpk[jt, :, 2048 + kt * P:2048 + (kt + 1) * P] = \
                wgT[kt * P:(kt + 1) * P, H + jt * P:H + (jt + 1) * P]
    # bias pack: [128, 16]; column layout r(4) z(4) i(4) h(4), col jt holds
    # bias[jt*128:(jt+1)*128]
    bias = np.concatenate([
        b_gate[:H].reshape(JT, P).T,
        b_gate[H:].reshape(JT, P).T,
        b_i.reshape(JT, P).T,
        b_h.reshape(JT, P).T,
    ], axis=1).astype(np.float32)
    bias = np.ascontiguousarray(bias)

    in_maps = []
    for c in range(NCORES):
        sl = slice(c * BL, (c + 1) * BL)
        in_maps.append({
            "xT": np.ascontiguousarray(input[sl].T),
            "hT": np.ascontiguousarray(hidden[sl].T),
            "wpk": wpk,
            "bias": bias,
        })

    res = run_bass_kernel_spmd(
        nc, in_maps, list(range(NCORES)),
        trace=bool(int(os.environ.get("GRU_TRACE", "0"))),
    )
    out = np.empty((B, H), dtype=np.float32)
    for c in range(NCORES):
        out[c * BL:(c + 1) * BL, :] = res.results[c]["outT"].T
    if res.exec_time_ns is not None:
        kernel.last_exec_time_ns = res.exec_time_ns
        kernel.last_results = res
    return out


kernel.last_exec_time_ns = None
kernel.last_results = None



# revision 6
# speedup vs baseline: 1.3136x; 1.3136x over previous
"""GRUCell fused kernel for Trainium2, data-parallel over 8 NeuronCores.

Strategy (v2, mixed precision):
  - Shard batch (16384) across 8 cores -> 2048 rows/core; replicate weights.
  - r/z gates (2/3 of the FLOPs, sigmoid-compressed so fp8-tolerant) run as
    fp8e4 DoubleRow matmuls: 2 K-rows per PE cell per cycle = 2x tensor
    throughput.  Acts scaled x16, gate weights x512 (TRN e4m3 max 240);
    the 1/8192 unscale is folded into the sigmoid ACT instruction.
  - i/h gates + epilogue in bf16 (verified 9.8e-3 rel Fro error overall vs
    2e-2 budget).
  - All activations resident in SBUF (6 MB) so every LDWEIGHTS is amortized
    over 4 batch-tile matmuls, keeping the PE stream dense (HAM stays warm).
  - Per output j-tile the gate phases run r -> z -> hg -> ig with PSUM bank
    ping-pong (r/hg on banks A0-3, z/ig on B0-3); the epilogue of each phase
    overlaps the next phase's matmuls on ACT/DVE/GPSIMD.
  - h' = n + z*(h - n), n = tanh(i_g + r*h_g), epilogue ops in bf16
    (2x DVE rate), final combine to fp32 output.
"""

import os
import numpy as np
import ml_dtypes
from contextlib import ExitStack

import concourse.bass as bass
import concourse.tile as tile
from concourse import bacc, mybir
from concourse.bass_utils import run_bass_kernel_spmd

B, I, H = 16384, 512, 512
NCORES = 8
BL = B // NCORES          # 2048 rows per core
NB = 512                  # batch tile (matmul moving free dim)
NBT = BL // NB            # 4 batch tiles per core
P = 128                   # partitions
KT = I // P               # 4 k-tiles (128) over features, per of x/h
KC = 2                    # 2 DoubleRow k-chunks (256) per of x/h
KS = (I + H) // P         # 8 k-subtiles (128) across the r/z contraction
JT = H // P               # 4 output j-tiles per gate

ASCALE = 16.0             # fp8 activation scale
WSCALE = 512.0            # fp8 weight scale
INV_SCALE = 1.0 / (ASCALE * WSCALE)

FP32 = mybir.dt.float32
BF16 = mybir.dt.bfloat16
FP8 = mybir.dt.float8e4

_cache = {}


def build_gru_bass():
    """Build (once) the SPMD Bass program for one core's shard."""
    if "nc" in _cache:
        return _cache["nc"]

    nc = bacc.Bacc(
        "TRN2",
        target_bir_lowering=False,
        debug=False,
        enable_asserts=False,
        num_devices=NCORES,
    )

    # feature-major activations; bf16 copies for i/h gates + epilogue,
    # DoubleRow-packed fp8 copies (scaled x16) for the r/z gates.
    xb = nc.dram_tensor("xb", [I, BL], BF16, kind="ExternalInput").ap()
    hb = nc.dram_tensor("hb", [H, BL], BF16, kind="ExternalInput").ap()
    # [p, ks, b] = x.T[ks*128 + p, b] * 16  (ks = 4 sub-k-tiles)
    x8 = nc.dram_tensor("x8", [P, 2 * KC, BL], FP8, kind="ExternalInput").ap()
    h8 = nc.dram_tensor("h8", [P, 2 * KC, BL], FP8, kind="ExternalInput").ap()
    # bf16 weights for i/h gates: [jt, p, kt*128 + m] = W.T[kt*128+p, jt*128+m]
    wi = nc.dram_tensor("wi", [JT, P, I], BF16, kind="ExternalInput").ap()
    wh = nc.dram_tensor("wh", [JT, P, H], BF16, kind="ExternalInput").ap()
    # fp8 DoubleRow weights for r/z: [jt, p, ks, m] = Wg.T[ks*128+p, jt*128+m]*512
    wr = nc.dram_tensor("wr", [JT, P, KS, P], FP8, kind="ExternalInput").ap()
    wz = nc.dram_tensor("wz", [JT, P, KS, P], FP8, kind="ExternalInput").ap()
    # bias columns: 0..3 b_r per j-tile, 4..7 b_z, 8..11 b_i, 12..15 b_h
    bias = nc.dram_tensor("bias", [P, 16], FP32, kind="ExternalInput").ap()
    outT = nc.dram_tensor("outT", [H, BL], FP32, kind="ExternalOutput").ap()

    ADD = mybir.AluOpType.add
    MULT = mybir.AluOpType.mult
    SUB = mybir.AluOpType.subtract
    SIG = mybir.ActivationFunctionType.Sigmoid
    TANH = mybir.ActivationFunctionType.Tanh
    DR = mybir.MatmulPerfMode.DoubleRow

    with tile.TileContext(nc) as tc, ExitStack() as ctx:
        wpool = ctx.enter_context(tc.tile_pool(name="weights", bufs=1))
        apool = ctx.enter_context(tc.tile_pool(name="acts", bufs=1))
        ppool = ctx.enter_context(tc.tile_pool(name="psum", bufs=1, space="PSUM"))
        epool = ctx.enter_context(tc.tile_pool(name="epi", bufs=2))

        bias_s = wpool.tile([P, 16], FP32, tag="bias", name="bias_s")
        nc.sync.dma_start(out=bias_s[:], in_=bias[:, :])

        # ---- input DMAs, in first-use order ----
        wr_s = [None] * JT
        wz_s = [None] * JT
        wi_s = [None] * JT
        wh_s = [None] * JT

        def load_w(which, jt):
            if which == "r":
                wr_s[jt] = wpool.tile([P, KS, P], FP8, tag=f"wr{jt}",
                                      name=f"wr{jt}")
                nc.sync.dma_start(out=wr_s[jt][:], in_=wr[jt, :, :, :])
            elif which == "z":
                wz_s[jt] = wpool.tile([P, KS, P], FP8, tag=f"wz{jt}",
                                      name=f"wz{jt}")
                nc.sync.dma_start(out=wz_s[jt][:], in_=wz[jt, :, :, :])
            elif which == "i":
                wi_s[jt] = wpool.tile([P, I], BF16, tag=f"wi{jt}", name=f"wi{jt}")
                nc.sync.dma_start(out=wi_s[jt][:], in_=wi[jt, :, :])
            else:
                wh_s[jt] = wpool.tile([P, H], BF16, tag=f"wh{jt}", name=f"wh{jt}")
                nc.sync.dma_start(out=wh_s[jt][:], in_=wh[jt, :, :])

        # fp8 acts (r/z inputs) first: jt0's r-phase starts after ~0.7MB.
        x8_s = apool.tile([P, 2 * KC, BL], FP8, tag="x8", name="x8_s")
        h8_s = apool.tile([P, 2 * KC, BL], FP8, tag="h8", name="h8_s")
        load_w("r", 0)
        nc.sync.dma_start(out=x8_s[:, 0:2, :], in_=x8[:, 0:2, :])
        nc.sync.dma_start(out=h8_s[:, 0:2, :], in_=h8[:, 0:2, :])
        nc.sync.dma_start(out=x8_s[:, 2:4, :], in_=x8[:, 2:4, :])
        nc.sync.dma_start(out=h8_s[:, 2:4, :], in_=h8[:, 2:4, :])
        load_w("z", 0)
        # bf16 acts for hg/ig phases + epilogue
        load_w("h", 0)
        hb_s = [None] * KT
        for kt in range(KT):
            hb_s[kt] = apool.tile([P, BL], BF16, tag=f"hb{kt}", name=f"hb{kt}")
            nc.sync.dma_start(out=hb_s[kt][:], in_=hb[kt * P:(kt + 1) * P, :])
        load_w("i", 0)
        xb_s = [None] * KT
        for kt in range(KT):
            xb_s[kt] = apool.tile([P, BL], BF16, tag=f"xb{kt}", name=f"xb{kt}")
            nc.sync.dma_start(out=xb_s[kt][:], in_=xb[kt * P:(kt + 1) * P, :])
        for jt in range(1, JT):
            for which in ("r", "z", "h", "i"):
                load_w(which, jt)

        # DoubleRow k-chunk order: interleave x/h chunks to match DMA arrival.
        # chunk -> (acts tile, acts ks, weight ks); weight subtiles 0-3 cover
        # the x features of the gate contraction, 4-7 the h features.
        def dr_src(kc):
            return [(x8_s, 0, 0), (h8_s, 0, 4), (x8_s, 2, 2), (h8_s, 2, 6)][kc]

        # ---- main loop over output j-tiles ----
        for jt in range(JT):
            j0 = jt * P

            # phase r: fp8 DoubleRow, K = I+H = 4 chunks of 256 (banks A)
            r_ps = [None] * NBT
            for bt in range(NBT):
                r_ps[bt] = ppool.tile([P, NB], FP32, tag=f"psA{bt}",
                                      name=f"r_ps_{jt}_{bt}")
            for kc in range(2 * KC):
                act, ks, ws = dr_src(kc)
                lhsT = wr_s[jt][:, ws:ws + 2, :]
                for bt in range(NBT):
                    nc.tensor.matmul(
                        out=r_ps[bt][:], lhsT=lhsT,
                        rhs=act[:, ks:ks + 2, bass.ts(bt, NB)],
                        start=(kc == 0), stop=(kc == 2 * KC - 1),
                        perf_mode=DR)
            r_s = [None] * NBT
            for bt in range(NBT):
                r_s[bt] = epool.tile([P, NB], BF16, tag=f"r_s{bt}",
                                     name=f"r_s_{jt}_{bt}")
                nc.scalar.activation(out=r_s[bt][:], in_=r_ps[bt][:], func=SIG,
                                     bias=bias_s[:, jt:jt + 1], scale=INV_SCALE)

            # phase z: fp8 DoubleRow (banks B)
            z_ps = [None] * NBT
            for bt in range(NBT):
                z_ps[bt] = ppool.tile([P, NB], FP32, tag=f"psB{bt}",
                                      name=f"z_ps_{jt}_{bt}")
            for kc in range(2 * KC):
                act, ks, ws = dr_src(kc)
                lhsT = wz_s[jt][:, ws:ws + 2, :]
                for bt in range(NBT):
                    nc.tensor.matmul(
                        out=z_ps[bt][:], lhsT=lhsT,
                        rhs=act[:, ks:ks + 2, bass.ts(bt, NB)],
                        start=(kc == 0), stop=(kc == 2 * KC - 1),
                        perf_mode=DR)
            z_s = [None] * NBT
            for bt in range(NBT):
                z_s[bt] = epool.tile([P, NB], BF16, tag=f"z_s{bt}",
                                     name=f"z_s_{jt}_{bt}")
                nc.scalar.activation(out=z_s[bt][:], in_=z_ps[bt][:], func=SIG,
                                     bias=bias_s[:, 4 + jt:5 + jt],
                                     scale=INV_SCALE)

            # phase hg: bf16, K = H (banks A, freed by the r sigmoids)
            hg_ps = [None] * NBT
            for bt in range(NBT):
                hg_ps[bt] = ppool.tile([P, NB], FP32, tag=f"psA{bt}",
                                       name=f"hg_ps_{jt}_{bt}")
            for kt in range(KT):
                lhsT = wh_s[jt][:, kt * P:(kt + 1) * P]
                for bt in range(NBT):
                    nc.tensor.matmul(
                        out=hg_ps[bt][:], lhsT=lhsT,
                        rhs=hb_s[kt][:, bass.ts(bt, NB)],
                        start=(kt == 0), stop=(kt == KT - 1))
            # m = (h_gate + b_h) * r   (DVE, frees banks A for next jt's r)
            m = [None] * NBT
            for bt in range(NBT):
                m[bt] = epool.tile([P, NB], BF16, tag=f"m{bt}",
                                   name=f"m_{jt}_{bt}")
                nc.vector.scalar_tensor_tensor(
                    out=m[bt][:], in0=hg_ps[bt][:],
                    scalar=bias_s[:, 12 + jt:13 + jt],
                    in1=r_s[bt][:], op0=ADD, op1=MULT)

            # phase ig: bf16, K = I (banks B, freed by the z sigmoids)
            ig_ps = [None] * NBT
            for bt in range(NBT):
                ig_ps[bt] = ppool.tile([P, NB], FP32, tag=f"psB{bt}",
                                       name=f"ig_ps_{jt}_{bt}")
            for kt in range(KT):
                lhsT = wi_s[jt][:, kt * P:(kt + 1) * P]
                for bt in range(NBT):
                    nc.tensor.matmul(
                        out=ig_ps[bt][:], lhsT=lhsT,
                        rhs=xb_s[kt][:, bass.ts(bt, NB)],
                        start=(kt == 0), stop=(kt == KT - 1))

            # epilogue: s = (i_gate + b_i) + m; n = tanh(s);
            # h' = n + z*(h - n)
            for bt in range(NBT):
                bsl = bass.ts(bt, NB)
                s = epool.tile([P, NB], BF16, tag=f"s{bt}", name=f"s_{jt}_{bt}")
                nc.vector.scalar_tensor_tensor(
                    out=s[:], in0=ig_ps[bt][:],
                    scalar=bias_s[:, 8 + jt:9 + jt],
                    in1=m[bt][:], op0=ADD, op1=ADD)
                n = epool.tile([P, NB], BF16, tag=f"n{bt}", name=f"n_{jt}_{bt}")
                nc.scalar.activation(out=n[:], in_=s[:], func=TANH)
                d = epool.tile([P, NB], BF16, tag=f"d{bt}", name=f"d_{jt}_{bt}")
                nc.gpsimd.tensor_tensor(
                    out=d[:], in0=hb_s[jt][:, bsl], in1=n[:], op=SUB)
                e = epool.tile([P, NB], BF16, tag=f"e{bt}", name=f"e_{jt}_{bt}")
                nc.gpsimd.tensor_tensor(
                    out=e[:], in0=z_s[bt][:], in1=d[:], op=MULT)
                o = epool.tile([P, NB], FP32, tag=f"o{bt}", name=f"o_{jt}_{bt}")
                nc.vector.tensor_tensor(
                    out=o[:], in0=n[:], in1=e[:], op=ADD)
                nc.sync.dma_start(out=outT[j0:j0 + P, bsl], in_=o[:])

    nc.compile()
    _cache["nc"] = nc
    return nc


def _pack_weights(W_gate, b_gate, W_i, b_i, W_h, b_h):
    bf16 = ml_dtypes.bfloat16
    fp8 = ml_dtypes.float8_e4m3

    def pack_bf16(WT):  # [I, H] -> [JT, P, I] with [jt, p, kt*128+m]
        a = WT.reshape(KT, P, JT, P).transpose(2, 1, 0, 3).reshape(JT, P, I)
        return np.ascontiguousarray(a.astype(bf16))

    def pack_fp8(WT):   # [1024, 512] -> [JT, P, 8, P]
        a = np.clip(WT * WSCALE, -240.0, 240.0)
        a = a.reshape(KS, P, JT, P).transpose(2, 1, 0, 3)
        return np.ascontiguousarray(a.astype(fp8))

    wi = pack_bf16(W_i.T)
    wh = pack_bf16(W_h.T)
    wr = pack_fp8(W_gate[:H].T)
    wz = pack_fp8(W_gate[H:].T)
    biasp = np.concatenate([
        b_gate[:H].reshape(JT, P).T,
        b_gate[H:].reshape(JT, P).T,
        b_i.reshape(JT, P).T,
        b_h.reshape(JT, P).T,
    ], axis=1).astype(np.float32)
    return wi, wh, wr, wz, np.ascontiguousarray(biasp)


def kernel(input, hidden, W_gate, b_gate, W_i, b_i, W_h, b_h):
    input = np.asarray(input, dtype=np.float32)
    hidden = np.asarray(hidden, dtype=np.float32)
    W_gate = np.asarray(W_gate, dtype=np.float32)
    b_gate = np.asarray(b_gate, dtype=np.float32)
    W_i = np.asarray(W_i, dtype=np.float32)
    b_i = np.asarray(b_i, dtype=np.float32)
    W_h = np.asarray(W_h, dtype=np.float32)
    b_h = np.asarray(b_h, dtype=np.float32)

    nc = build_gru_bass()
    wi, wh, wr, wz, biasp = _pack_weights(W_gate, b_gate, W_i, b_i, W_h, b_h)

    bf16 = ml_dtypes.bfloat16
    fp8 = ml_dtypes.float8_e4m3

    def pack8(aT):  # [512, BL] fp32 -> [P, 4, BL] fp8 (scaled)
        a = np.clip(aT * ASCALE, -240.0, 240.0)
        a = a.reshape(2 * KC, P, BL).transpose(1, 0, 2)
        return np.ascontiguousarray(a.astype(fp8))

    in_maps = []
    for c in range(NCORES):
        sl = slice(c * BL, (c + 1) * BL)
        xT = np.ascontiguousarray(input[sl].T)
        hT = np.ascontiguousarray(hidden[sl].T)
        in_maps.append({
            "xb": np.ascontiguousarray(xT.astype(bf16)),
            "hb": np.ascontiguousarray(hT.astype(bf16)),
            "x8": pack8(xT),
            "h8": pack8(hT),
            "wi": wi, "wh": wh, "wr": wr, "wz": wz,
            "bias": biasp,
        })

    res = run_bass_kernel_spmd(
        nc, in_maps, list(range(NCORES)),
        trace=bool(int(os.environ.get("GRU_TRACE", "0"))),
    )
    out = np.empty((B, H), dtype=np.float32)
    for c in range(NCORES):
        out[c * BL:(c + 1) * BL, :] = res.results[c]["outT"].T
    if res.exec_time_ns is not None:
        kernel.last_exec_time_ns = res.exec_time_ns
        kernel.last_results = res
    return out


kernel.last_exec_time_ns = None
kernel.last_results = None


# revision 7
# speedup vs baseline: 1.5291x; 1.1640x over previous
"""GRUCell fused kernel for Trainium2, data-parallel over 8 NeuronCores.

Strategy (v3, mixed precision):
  - Shard batch (16384) across 8 cores -> 2048 rows/core; replicate weights.
  - r/z/h gates run as fp8e4 DoubleRow matmuls: 2 K-rows per PE cell per
    cycle = 2x tensor throughput.  Acts scaled x16, weights x512 (TRN e4m3
    max 240); the 1/8192 unscale is folded into the PSUM-reading ACT op.
    i gate + epilogue in bf16.  Whole-scheme numerics verified at 1.24e-2
    rel Fro error vs the 2e-2 budget (hardware matched the numpy model to
    4 digits on the previous variant).
  - All activations resident in SBUF so the PE stream never waits on acts;
    per output j-tile the phases run r -> z -> hg -> ig with PSUM bank
    ping-pong (r/hg on banks A0-3, z/ig on B0-3).
  - Batch-tile-outer matmul loops: each bank's accumulation completes at
    1/4 of the phase, so the ACT/DVE epilogue pipelines into the MM stream
    and the post-stream tail is one tile's chain (~3us).
  - Epilogue per tile: r=sig(r_ps/8192+br), z=sig(z_ps/8192+bz) [ACT],
    hgm=(hg_ps/8192+bh) [ACT Identity], m=hgm*r, s=(ig_ps+bi)+m [DVE],
    n=tanh(s) [ACT], d=h-n, e=z*d, o=n+e [DVE, all bf16 = 2x rate].
  - bf16 output (host upcasts); in-DMA ~7.8MB, out 2MB.
"""

import os
import numpy as np
import ml_dtypes
from contextlib import ExitStack

import concourse.bass as bass
import concourse.tile as tile
from concourse import bacc, mybir
from concourse.bass_utils import run_bass_kernel_spmd

B, I, H = 16384, 512, 512
NCORES = 8
BL = B // NCORES          # 2048 rows per core
NB = 512                  # batch tile (matmul moving free dim)
NBT = BL // NB            # 4 batch tiles per core
P = 128                   # partitions
KT = I // P               # 4 k-tiles (128) per of x/h
KS = (I + H) // P         # 8 k-subtiles across the r/z contraction
JT = H // P               # 4 output j-tiles per gate

ASCALE = 16.0             # fp8 activation scale
WSCALE = 512.0            # fp8 weight scale
INV_SCALE = 1.0 / (ASCALE * WSCALE)

FP32 = mybir.dt.float32
BF16 = mybir.dt.bfloat16
FP8 = mybir.dt.float8e4

_cache = {}


def build_gru_bass():
    """Build (once) the SPMD Bass program for one core's shard."""
    if "nc" in _cache:
        return _cache["nc"]

    nc = bacc.Bacc(
        "TRN2",
        target_bir_lowering=False,
        debug=False,
        enable_asserts=False,
        num_devices=NCORES,
    )

    # feature-major activations: bf16 x for the i gate, bf16 h for the
    # epilogue interpolation, DoubleRow-packed scaled fp8 x/h for r/z/hg.
    xb = nc.dram_tensor("xb", [I, BL], BF16, kind="ExternalInput").ap()
    hb = nc.dram_tensor("hb", [H, BL], BF16, kind="ExternalInput").ap()
    x8 = nc.dram_tensor("x8", [P, 4, BL], FP8, kind="ExternalInput").ap()
    h8 = nc.dram_tensor("h8", [P, 4, BL], FP8, kind="ExternalInput").ap()
    # bf16 weights for the i gate
    wi = nc.dram_tensor("wi", [JT, P, I], BF16, kind="ExternalInput").ap()
    # fp8 DoubleRow weights: [jt, p, ks, m] = W.T[ks*128+p, jt*128+m]*512
    wr = nc.dram_tensor("wr", [JT, P, KS, P], FP8, kind="ExternalInput").ap()
    wz = nc.dram_tensor("wz", [JT, P, KS, P], FP8, kind="ExternalInput").ap()
    wh = nc.dram_tensor("wh", [JT, P, KT, P], FP8, kind="ExternalInput").ap()
    # bias columns: 0..3 b_r per j-tile, 4..7 b_z, 8..11 b_i, 12..15 b_h
    bias = nc.dram_tensor("bias", [P, 16], FP32, kind="ExternalInput").ap()
    outT = nc.dram_tensor("outT", [H, BL], BF16, kind="ExternalOutput").ap()

    ADD = mybir.AluOpType.add
    MULT = mybir.AluOpType.mult
    SUB = mybir.AluOpType.subtract
    SIG = mybir.ActivationFunctionType.Sigmoid
    TANH = mybir.ActivationFunctionType.Tanh
    IDENT = mybir.ActivationFunctionType.Identity
    DR = mybir.MatmulPerfMode.DoubleRow

    with tile.TileContext(nc) as tc, ExitStack() as ctx:
        wpool = ctx.enter_context(tc.tile_pool(name="weights", bufs=1))
        apool = ctx.enter_context(tc.tile_pool(name="acts", bufs=1))
        ppool = ctx.enter_context(tc.tile_pool(name="psum", bufs=1, space="PSUM"))
        epool = ctx.enter_context(tc.tile_pool(name="epi", bufs=2))

        bias_s = wpool.tile([P, 16], FP32, tag="bias", name="bias_s")
        nc.sync.dma_start(out=bias_s[:], in_=bias[:, :])

        wr_s = [None] * JT
        wz_s = [None] * JT
        wh_s = [None] * JT
        wi_s = [None] * JT

        def load_w(which, jt):
            if which == "r":
                wr_s[jt] = wpool.tile([P, KS, P], FP8, tag=f"wr{jt}",
                                      name=f"wr{jt}")
                nc.sync.dma_start(out=wr_s[jt][:], in_=wr[jt, :, :, :])
            elif which == "z":
                wz_s[jt] = wpool.tile([P, KS, P], FP8, tag=f"wz{jt}",
                                      name=f"wz{jt}")
                nc.sync.dma_start(out=wz_s[jt][:], in_=wz[jt, :, :, :])
            elif which == "h":
                wh_s[jt] = wpool.tile([P, KT, P], FP8, tag=f"wh{jt}",
                                      name=f"wh{jt}")
                nc.sync.dma_start(out=wh_s[jt][:], in_=wh[jt, :, :, :])
            else:
                wi_s[jt] = wpool.tile([P, I], BF16, tag=f"wi{jt}", name=f"wi{jt}")
                nc.sync.dma_start(out=wi_s[jt][:], in_=wi[jt, :, :])

        # ---- input DMAs, in first-use order ----
        x8_s = apool.tile([P, 4, BL], FP8, tag="x8", name="x8_s")
        h8_s = apool.tile([P, 4, BL], FP8, tag="h8", name="h8_s")
        load_w("r", 0)
        nc.sync.dma_start(out=x8_s[:, 0:2, :], in_=x8[:, 0:2, :])
        nc.sync.dma_start(out=x8_s[:, 2:4, :], in_=x8[:, 2:4, :])
        nc.sync.dma_start(out=h8_s[:, 0:2, :], in_=h8[:, 0:2, :])
        nc.sync.dma_start(out=h8_s[:, 2:4, :], in_=h8[:, 2:4, :])
        load_w("z", 0)
        load_w("h", 0)
        hb_s = [None] * KT
        for kt in range(KT):
            hb_s[kt] = apool.tile([P, BL], BF16, tag=f"hb{kt}", name=f"hb{kt}")
            nc.sync.dma_start(out=hb_s[kt][:], in_=hb[kt * P:(kt + 1) * P, :])
        load_w("i", 0)
        xb_s = [None] * KT
        for kt in range(KT):
            xb_s[kt] = apool.tile([P, BL], BF16, tag=f"xb{kt}", name=f"xb{kt}")
            nc.sync.dma_start(out=xb_s[kt][:], in_=xb[kt * P:(kt + 1) * P, :])
        for jt in range(1, JT):
            for which in ("r", "z", "h", "i"):
                load_w(which, jt)

        # r/z DoubleRow chunks in DMA-arrival order: x first, then h.
        # chunk -> (acts tile, acts ks, weight ks)
        RZ_CHUNKS = [(x8_s, 0, 0), (x8_s, 2, 2), (h8_s, 0, 4), (h8_s, 2, 6)]
        HG_CHUNKS = [(h8_s, 0, 0), (h8_s, 2, 2)]

        def dr_phase(ps, w_t, chunks, bt_outer):
            nck = len(chunks)
            if bt_outer:
                for bt in range(NBT):
                    for kc in range(nck):
                        act, ks, ws = chunks[kc]
                        nc.tensor.matmul(
                            out=ps[bt][:], lhsT=w_t[:, ws:ws + 2, :],
                            rhs=act[:, ks:ks + 2, bass.ts(bt, NB)],
                            start=(kc == 0), stop=(kc == nck - 1),
                            perf_mode=DR)
            else:
                for kc in range(nck):
                    act, ks, ws = chunks[kc]
                    for bt in range(NBT):
                        nc.tensor.matmul(
                            out=ps[bt][:], lhsT=w_t[:, ws:ws + 2, :],
                            rhs=act[:, ks:ks + 2, bass.ts(bt, NB)],
                            start=(kc == 0), stop=(kc == nck - 1),
                            perf_mode=DR)

        def bf_phase(ps, w_t, acts, bt_outer):
            if bt_outer:
                for bt in range(NBT):
                    for kt in range(KT):
                        nc.tensor.matmul(
                            out=ps[bt][:], lhsT=w_t[:, kt * P:(kt + 1) * P],
                            rhs=acts[kt][:, bass.ts(bt, NB)],
                            start=(kt == 0), stop=(kt == KT - 1))
            else:
                for kt in range(KT):
                    for bt in range(NBT):
                        nc.tensor.matmul(
                            out=ps[bt][:], lhsT=w_t[:, kt * P:(kt + 1) * P],
                            rhs=acts[kt][:, bass.ts(bt, NB)],
                            start=(kt == 0), stop=(kt == KT - 1))

        # ---- main loop over output j-tiles ----
        for jt in range(JT):
            j0 = jt * P
            first = jt == 0

            # phase r: fp8 DR, K = I+H (banks A)
            r_ps = [ppool.tile([P, NB], FP32, tag=f"psA{bt}",
                               name=f"r_ps_{jt}_{bt}") for bt in range(NBT)]
            dr_phase(r_ps, wr_s[jt], RZ_CHUNKS, bt_outer=not first)
            r_s = [None] * NBT
            for bt in range(NBT):
                r_s[bt] = epool.tile([P, NB], BF16, tag=f"r_s{bt}",
                                     name=f"r_s_{jt}_{bt}")
                nc.scalar.activation(out=r_s[bt][:], in_=r_ps[bt][:], func=SIG,
                                     bias=bias_s[:, jt:jt + 1], scale=INV_SCALE)

            # phase z: fp8 DR (banks B)
            z_ps = [ppool.tile([P, NB], FP32, tag=f"psB{bt}",
                               name=f"z_ps_{jt}_{bt}") for bt in range(NBT)]
            dr_phase(z_ps, wz_s[jt], RZ_CHUNKS, bt_outer=not first)
            z_s = [None] * NBT
            for bt in range(NBT):
                z_s[bt] = epool.tile([P, NB], BF16, tag=f"z_s{bt}",
                                     name=f"z_s_{jt}_{bt}")
                nc.scalar.activation(out=z_s[bt][:], in_=z_ps[bt][:], func=SIG,
                                     bias=bias_s[:, 4 + jt:5 + jt],
                                     scale=INV_SCALE)

            # phase hg: fp8 DR, K = H (banks A, freed by the r sigmoids)
            hg_ps = [ppool.tile([P, NB], FP32, tag=f"psA{bt}",
                                name=f"hg_ps_{jt}_{bt}") for bt in range(NBT)]
            dr_phase(hg_ps, wh_s[jt], HG_CHUNKS, bt_outer=not first)
            # hgm = hg/8192 + b_h (ACT move, frees banks A); m = hgm * r
            m = [None] * NBT
            for bt in range(NBT):
                hgm = epool.tile([P, NB], BF16, tag=f"hgm{bt}",
                                 name=f"hgm_{jt}_{bt}")
                nc.scalar.activation(out=hgm[:], in_=hg_ps[bt][:], func=IDENT,
                                     bias=bias_s[:, 12 + jt:13 + jt],
                                     scale=INV_SCALE)
                m[bt] = epool.tile([P, NB], BF16, tag=f"m{bt}",
                                   name=f"m_{jt}_{bt}")
                nc.vector.tensor_tensor(out=m[bt][:], in0=hgm[:],
                                        in1=r_s[bt][:], op=MULT)

            # phase ig: bf16, K = I (banks B, freed by the z sigmoids)
            ig_ps = [ppool.tile([P, NB], FP32, tag=f"psB{bt}",
                                name=f"ig_ps_{jt}_{bt}") for bt in range(NBT)]
            bf_phase(ig_ps, wi_s[jt], xb_s, bt_outer=not first)

            # epilogue: s = (i_gate + b_i) + m; n = tanh(s); h' = n + z*(h-n)
            for bt in range(NBT):
                bsl = bass.ts(bt, NB)
                s = epool.tile([P, NB], BF16, tag=f"s{bt}", name=f"s_{jt}_{bt}")
                nc.vector.scalar_tensor_tensor(
                    out=s[:], in0=ig_ps[bt][:],
                    scalar=bias_s[:, 8 + jt:9 + jt],
                    in1=m[bt][:], op0=ADD, op1=ADD)
                n = epool.tile([P, NB], BF16, tag=f"n{bt}", name=f"n_{jt}_{bt}")
                nc.scalar.activation(out=n[:], in_=s[:], func=TANH)
                d = epool.tile([P, NB], BF16, tag=f"d{bt}", name=f"d_{jt}_{bt}")
                nc.vector.tensor_tensor(
                    out=d[:], in0=hb_s[jt][:, bsl], in1=n[:], op=SUB)
                e = epool.tile([P, NB], BF16, tag=f"e{bt}", name=f"e_{jt}_{bt}")
                nc.vector.tensor_tensor(
                    out=e[:], in0=z_s[bt][:], in1=d[:], op=MULT)
                o = epool.tile([P, NB], BF16, tag=f"o{bt}", name=f"o_{jt}_{bt}")
                nc.vector.tensor_tensor(
                    out=o[:], in0=n[:], in1=e[:], op=ADD)
                nc.sync.dma_start(out=outT[j0:j0 + P, bsl], in_=o[:])

    nc.compile()
    _cache["nc"] = nc
    return nc


def _pack_weights(W_gate, b_gate, W_i, b_i, W_h, b_h):
    bf16 = ml_dtypes.bfloat16
    fp8 = ml_dtypes.float8_e4m3

    def pack_bf16(WT):  # [I, H] -> [JT, P, I] with [jt, p, kt*128+m]
        a = WT.reshape(KT, P, JT, P).transpose(2, 1, 0, 3).reshape(JT, P, I)
        return np.ascontiguousarray(a.astype(bf16))

    def pack_fp8(WT):   # [K, 512] -> [JT, P, K/128, P]
        ks = WT.shape[0] // P
        a = np.clip(WT * WSCALE, -240.0, 240.0)
        a = a.reshape(ks, P, JT, P).transpose(2, 1, 0, 3)
        return np.ascontiguousarray(a.astype(fp8))

    wi = pack_bf16(W_i.T)
    wr = pack_fp8(W_gate[:H].T)
    wz = pack_fp8(W_gate[H:].T)
    wh = pack_fp8(W_h.T)
    biasp = np.concatenate([
        b_gate[:H].reshape(JT, P).T,
        b_gate[H:].reshape(JT, P).T,
        b_i.reshape(JT, P).T,
        b_h.reshape(JT, P).T,
    ], axis=1).astype(np.float32)
    return wi, wr, wz, wh, np.ascontiguousarray(biasp)


def kernel(input, hidden, W_gate, b_gate, W_i, b_i, W_h, b_h):
    input = np.asarray(input, dtype=np.float32)
    hidden = np.asarray(hidden, dtype=np.float32)
    W_gate = np.asarray(W_gate, dtype=np.float32)
    b_gate = np.asarray(b_gate, dtype=np.float32)
    W_i = np.asarray(W_i, dtype=np.float32)
    b_i = np.asarray(b_i, dtype=np.float32)
    W_h = np.asarray(W_h, dtype=np.float32)
    b_h = np.asarray(b_h, dtype=np.float32)

    nc = build_gru_bass()
    wi, wr, wz, wh, biasp = _pack_weights(W_gate, b_gate, W_i, b_i, W_h, b_h)

    bf16 = ml_dtypes.bfloat16
    fp8 = ml_dtypes.float8_e4m3

    def pack8(aT):  # [512, BL] fp32 -> [P, 4, BL] fp8 (scaled)
        a = np.clip(aT * ASCALE, -240.0, 240.0)
        a = a.reshape(4, P, BL).transpose(1, 0, 2)
        return np.ascontiguousarray(a.astype(fp8))

    in_maps = []
    for c in range(NCORES):
        sl = slice(c * BL, (c + 1) * BL)
        xT = np.ascontiguousarray(input[sl].T)
        hT = np.ascontiguousarray(hidden[sl].T)
        in_maps.append({
            "xb": np.ascontiguousarray(xT.astype(bf16)),
            "hb": np.ascontiguousarray(hT.astype(bf16)),
            "x8": pack8(xT),
            "h8": pack8(hT),
            "wi": wi, "wr": wr, "wz": wz, "wh": wh,
            "bias": biasp,
        })

    res = run_bass_kernel_spmd(
        nc, in_maps, list(range(NCORES)),
        trace=bool(int(os.environ.get("GRU_TRACE", "0"))),
    )
    out = np.empty((B, H), dtype=np.float32)
    for c in range(NCORES):
        out[c * BL:(c + 1) * BL, :] = res.results[c]["outT"].astype(np.float32).T
    if res.exec_time_ns is not None:
        kernel.last_exec_time_ns = res.exec_time_ns
        kernel.last_results = res
    return out


kernel.last_exec_time_ns = None
kernel.last_results = None


# revision 8
# speedup vs baseline: 1.6104x; 1.0532x over previous
"""GRUCell fused kernel for Trainium2, data-parallel over 8 NeuronCores.

Strategy (v3, mixed precision):
  - Shard batch (16384) across 8 cores -> 2048 rows/core; replicate weights.
  - r/z/h gates run as fp8e4 DoubleRow matmuls: 2 K-rows per PE cell per
    cycle = 2x tensor throughput.  Acts scaled x16, weights x512 (TRN e4m3
    max 240); the 1/8192 unscale is folded into the PSUM-reading ACT op.
    i gate + epilogue in bf16.  Whole-scheme numerics verified at 1.24e-2
    rel Fro error vs the 2e-2 budget (hardware matched the numpy model to
    4 digits on the previous variant).
  - All activations resident in SBUF so the PE stream never waits on acts;
    per output j-tile the phases run r -> z -> hg -> ig with PSUM bank
    ping-pong (r/hg on banks A0-3, z/ig on B0-3).
  - Batch-tile-outer matmul loops: each bank's accumulation completes at
    1/4 of the phase, so the ACT/DVE epilogue pipelines into the MM stream
    and the post-stream tail is one tile's chain (~3us).
  - Epilogue per tile: r=sig(r_ps/8192+br), z=sig(z_ps/8192+bz) [ACT],
    hgm=(hg_ps/8192+bh) [ACT Identity], m=hgm*r, s=(ig_ps+bi)+m [DVE],
    n=tanh(s) [ACT], d=h-n, e=z*d, o=n+e [DVE, all bf16 = 2x rate].
  - bf16 output (host upcasts); in-DMA ~7.8MB, out 2MB.
"""

import os
import numpy as np
import ml_dtypes
from contextlib import ExitStack

import concourse.bass as bass
import concourse.tile as tile
from concourse import bacc, mybir
from concourse.bass_utils import run_bass_kernel_spmd

B, I, H = 16384, 512, 512
NCORES = 8
BL = B // NCORES          # 2048 rows per core
NB = 512                  # batch tile (matmul moving free dim)
NBT = BL // NB            # 4 batch tiles per core
P = 128                   # partitions
KT = I // P               # 4 k-tiles (128) per of x/h
KS = (I + H) // P         # 8 k-subtiles across the r/z contraction
JT = H // P               # 4 output j-tiles per gate

ASCALE = 16.0             # fp8 activation scale
WSCALE = 512.0            # fp8 weight scale
INV_SCALE = 1.0 / (ASCALE * WSCALE)

FP32 = mybir.dt.float32
BF16 = mybir.dt.bfloat16
FP8 = mybir.dt.float8e4

_cache = {}


def build_gru_bass():
    """Build (once) the SPMD Bass program for one core's shard."""
    if "nc" in _cache:
        return _cache["nc"]

    nc = bacc.Bacc(
        "TRN2",
        target_bir_lowering=False,
        debug=False,
        enable_asserts=False,
        num_devices=NCORES,
    )

    # feature-major activations: bf16 x for the i gate, bf16 h for the
    # epilogue interpolation, DoubleRow-packed scaled fp8 x/h for r/z/hg.
    xb = nc.dram_tensor("xb", [I, BL], BF16, kind="ExternalInput").ap()
    hb = nc.dram_tensor("hb", [H, BL], BF16, kind="ExternalInput").ap()
    x8 = nc.dram_tensor("x8", [P, 4, BL], FP8, kind="ExternalInput").ap()
    h8 = nc.dram_tensor("h8", [P, 4, BL], FP8, kind="ExternalInput").ap()
    # bf16 weights for the i gate
    wi = nc.dram_tensor("wi", [JT, P, I], BF16, kind="ExternalInput").ap()
    # fp8 DoubleRow weights: [jt, p, ks, m] = W.T[ks*128+p, jt*128+m]*512
    wr = nc.dram_tensor("wr", [JT, P, KS, P], FP8, kind="ExternalInput").ap()
    wz = nc.dram_tensor("wz", [JT, P, KS, P], FP8, kind="ExternalInput").ap()
    wh = nc.dram_tensor("wh", [JT, P, KT, P], FP8, kind="ExternalInput").ap()
    # bias columns: 0..3 b_r per j-tile, 4..7 b_z, 8..11 b_i, 12..15 b_h
    bias = nc.dram_tensor("bias", [P, 16], FP32, kind="ExternalInput").ap()
    outT = nc.dram_tensor("outT", [H, BL], BF16, kind="ExternalOutput").ap()

    ADD = mybir.AluOpType.add
    MULT = mybir.AluOpType.mult
    SUB = mybir.AluOpType.subtract
    SIG = mybir.ActivationFunctionType.Sigmoid
    TANH = mybir.ActivationFunctionType.Tanh
    IDENT = mybir.ActivationFunctionType.Identity
    DR = mybir.MatmulPerfMode.DoubleRow

    with tile.TileContext(nc) as tc, ExitStack() as ctx:
        wpool = ctx.enter_context(tc.tile_pool(name="weights", bufs=1))
        apool = ctx.enter_context(tc.tile_pool(name="acts", bufs=1))
        ppool = ctx.enter_context(tc.tile_pool(name="psum", bufs=1, space="PSUM"))
        epool = ctx.enter_context(tc.tile_pool(name="epi", bufs=2))

        bias_s = wpool.tile([P, 16], FP32, tag="bias", name="bias_s")
        nc.sync.dma_start(out=bias_s[:], in_=bias[:, :])

        wr_s = [None] * JT
        wz_s = [None] * JT
        wh_s = [None] * JT
        wi_s = [None] * JT

        def load_w(which, jt):
            if which == "r":
                wr_s[jt] = wpool.tile([P, KS, P], FP8, tag=f"wr{jt}",
                                      name=f"wr{jt}")
                nc.sync.dma_start(out=wr_s[jt][:], in_=wr[jt, :, :, :])
            elif which == "z":
                wz_s[jt] = wpool.tile([P, KS, P], FP8, tag=f"wz{jt}",
                                      name=f"wz{jt}")
                nc.sync.dma_start(out=wz_s[jt][:], in_=wz[jt, :, :, :])
            elif which == "h":
                wh_s[jt] = wpool.tile([P, KT, P], FP8, tag=f"wh{jt}",
                                      name=f"wh{jt}")
                nc.sync.dma_start(out=wh_s[jt][:], in_=wh[jt, :, :, :])
            else:
                wi_s[jt] = wpool.tile([P, I], BF16, tag=f"wi{jt}", name=f"wi{jt}")
                nc.sync.dma_start(out=wi_s[jt][:], in_=wi[jt, :, :])

        # ---- input DMAs, in first-use order ----
        x8_s = apool.tile([P, 4, BL], FP8, tag="x8", name="x8_s")
        h8_s = apool.tile([P, 4, BL], FP8, tag="h8", name="h8_s")
        load_w("r", 0)
        nc.sync.dma_start(out=x8_s[:, 0:2, :], in_=x8[:, 0:2, :])
        nc.sync.dma_start(out=x8_s[:, 2:4, :], in_=x8[:, 2:4, :])
        nc.sync.dma_start(out=h8_s[:, 0:2, :], in_=h8[:, 0:2, :])
        nc.sync.dma_start(out=h8_s[:, 2:4, :], in_=h8[:, 2:4, :])
        load_w("h", 0)
        load_w("i", 0)
        xb_s = [None] * KT
        for kt in range(KT):
            xb_s[kt] = apool.tile([P, BL], BF16, tag=f"xb{kt}", name=f"xb{kt}")
            nc.sync.dma_start(out=xb_s[kt][:], in_=xb[kt * P:(kt + 1) * P, :])
        load_w("z", 0)
        hb_s = [None] * KT
        for kt in range(KT):
            hb_s[kt] = apool.tile([P, BL], BF16, tag=f"hb{kt}", name=f"hb{kt}")
            nc.sync.dma_start(out=hb_s[kt][:], in_=hb[kt * P:(kt + 1) * P, :])
        for jt in range(1, JT):
            for which in ("r", "h", "i", "z"):
                load_w(which, jt)

        # r/z DoubleRow chunks in DMA-arrival order: x first, then h.
        # chunk -> (acts tile, acts ks, weight ks)
        RZ_CHUNKS = [(x8_s, 0, 0), (x8_s, 2, 2), (h8_s, 0, 4), (h8_s, 2, 6)]
        HG_CHUNKS = [(h8_s, 0, 0), (h8_s, 2, 2)]

        def dr_phase(ps, w_t, chunks, bt_outer):
            nck = len(chunks)
            if bt_outer:
                for bt in range(NBT):
                    for kc in range(nck):
                        act, ks, ws = chunks[kc]
                        nc.tensor.matmul(
                            out=ps[bt][:], lhsT=w_t[:, ws:ws + 2, :],
                            rhs=act[:, ks:ks + 2, bass.ts(bt, NB)],
                            start=(kc == 0), stop=(kc == nck - 1),
                            perf_mode=DR)
            else:
                for kc in range(nck):
                    act, ks, ws = chunks[kc]
                    for bt in range(NBT):
                        nc.tensor.matmul(
                            out=ps[bt][:], lhsT=w_t[:, ws:ws + 2, :],
                            rhs=act[:, ks:ks + 2, bass.ts(bt, NB)],
                            start=(kc == 0), stop=(kc == nck - 1),
                            perf_mode=DR)

        def bf_phase(ps, w_t, acts, bt_outer):
            if bt_outer:
                for bt in range(NBT):
                    for kt in range(KT):
                        nc.tensor.matmul(
                            out=ps[bt][:], lhsT=w_t[:, kt * P:(kt + 1) * P],
                            rhs=acts[kt][:, bass.ts(bt, NB)],
                            start=(kt == 0), stop=(kt == KT - 1))
            else:
                for kt in range(KT):
                    for bt in range(NBT):
                        nc.tensor.matmul(
                            out=ps[bt][:], lhsT=w_t[:, kt * P:(kt + 1) * P],
                            rhs=acts[kt][:, bass.ts(bt, NB)],
                            start=(kt == 0), stop=(kt == KT - 1))

        # ---- main loop over output j-tiles ----
        for jt in range(JT):
            j0 = jt * P
            first = jt == 0

            # phase r: fp8 DR, K = I+H (banks A)
            r_ps = [ppool.tile([P, NB], FP32, tag=f"psA{bt}",
                               name=f"r_ps_{jt}_{bt}") for bt in range(NBT)]
            dr_phase(r_ps, wr_s[jt], RZ_CHUNKS, bt_outer=not first)
            r_s = [None] * NBT
            for bt in range(NBT):
                r_s[bt] = epool.tile([P, NB], BF16, tag=f"r_s{bt}",
                                     name=f"r_s_{jt}_{bt}")
                nc.scalar.activation(out=r_s[bt][:], in_=r_ps[bt][:], func=SIG,
                                     bias=bias_s[:, jt:jt + 1], scale=INV_SCALE)

            # phase hg: fp8 DR, K = H (banks A, freed per-bank by the r
            # sigmoids)
            hg_ps = [ppool.tile([P, NB], FP32, tag=f"psA{bt}",
                                name=f"hg_ps_{jt}_{bt}") for bt in range(NBT)]
            dr_phase(hg_ps, wh_s[jt], HG_CHUNKS, bt_outer=True)
            # hgm = hg/8192 + b_h (ACT move, frees banks A); m = hgm * r
            m = [None] * NBT
            for bt in range(NBT):
                hgm = epool.tile([P, NB], BF16, tag=f"hgm{bt}",
                                 name=f"hgm_{jt}_{bt}")
                nc.scalar.activation(out=hgm[:], in_=hg_ps[bt][:], func=IDENT,
                                     bias=bias_s[:, 12 + jt:13 + jt],
                                     scale=INV_SCALE)
                m[bt] = epool.tile([P, NB], BF16, tag=f"m{bt}",
                                   name=f"m_{jt}_{bt}")
                nc.vector.tensor_tensor(out=m[bt][:], in0=hgm[:],
                                        in1=r_s[bt][:], op=MULT)

            # phase ig: bf16, K = I (banks B)
            ig_ps = [ppool.tile([P, NB], FP32, tag=f"psB{bt}",
                                name=f"ig_ps_{jt}_{bt}") for bt in range(NBT)]
            bf_phase(ig_ps, wi_s[jt], xb_s, bt_outer=not first)
            # s = (i_gate + b_i) + m; n = tanh(s); d = h - n  (frees banks B)
            n = [None] * NBT
            d = [None] * NBT
            for bt in range(NBT):
                bsl = bass.ts(bt, NB)
                s = epool.tile([P, NB], BF16, tag=f"s{bt}", name=f"s_{jt}_{bt}")
                nc.vector.scalar_tensor_tensor(
                    out=s[:], in0=ig_ps[bt][:],
                    scalar=bias_s[:, 8 + jt:9 + jt],
                    in1=m[bt][:], op0=ADD, op1=ADD)
                n[bt] = epool.tile([P, NB], BF16, tag=f"n{bt}",
                                   name=f"n_{jt}_{bt}")
                nc.scalar.activation(out=n[bt][:], in_=s[:], func=TANH)
                d[bt] = epool.tile([P, NB], BF16, tag=f"d{bt}",
                                   name=f"d_{jt}_{bt}")
                nc.vector.tensor_tensor(
                    out=d[bt][:], in0=hb_s[jt][:, bsl], in1=n[bt][:], op=SUB)

            # phase z: fp8 DR (banks B, freed per-bank by the s ops); its
            # epilogue is the shortest chain, so it goes last.
            z_ps = [ppool.tile([P, NB], FP32, tag=f"psB{bt}",
                               name=f"z_ps_{jt}_{bt}") for bt in range(NBT)]
            dr_phase(z_ps, wz_s[jt], RZ_CHUNKS, bt_outer=True)
            for bt in range(NBT):
                bsl = bass.ts(bt, NB)
                z_s = epool.tile([P, NB], BF16, tag=f"z_s{bt}",
                                 name=f"z_s_{jt}_{bt}")
                nc.scalar.activation(out=z_s[:], in_=z_ps[bt][:], func=SIG,
                                     bias=bias_s[:, 4 + jt:5 + jt],
                                     scale=INV_SCALE)
                e = epool.tile([P, NB], BF16, tag=f"e{bt}", name=f"e_{jt}_{bt}")
                nc.vector.tensor_tensor(
                    out=e[:], in0=z_s[:], in1=d[bt][:], op=MULT)
                o = epool.tile([P, NB], BF16, tag=f"o{bt}", name=f"o_{jt}_{bt}")
                nc.vector.tensor_tensor(
                    out=o[:], in0=n[bt][:], in1=e[:], op=ADD)
                nc.sync.dma_start(out=outT[j0:j0 + P, bsl], in_=o[:])

    nc.compile()
    _cache["nc"] = nc
    return nc


def _pack_weights(W_gate, b_gate, W_i, b_i, W_h, b_h):
    bf16 = ml_dtypes.bfloat16
    fp8 = ml_dtypes.float8_e4m3

    def pack_bf16(WT):  # [I, H] -> [JT, P, I] with [jt, p, kt*128+m]
        a = WT.reshape(KT, P, JT, P).transpose(2, 1, 0, 3).reshape(JT, P, I)
        return np.ascontiguousarray(a.astype(bf16))

    def pack_fp8(WT):   # [K, 512] -> [JT, P, K/128, P]
        ks = WT.shape[0] // P
        a = np.clip(WT * WSCALE, -240.0, 240.0)
        a = a.reshape(ks, P, JT, P).transpose(2, 1, 0, 3)
        return np.ascontiguousarray(a.astype(fp8))

    wi = pack_bf16(W_i.T)
    wr = pack_fp8(W_gate[:H].T)
    wz = pack_fp8(W_gate[H:].T)
    wh = pack_fp8(W_h.T)
    biasp = np.concatenate([
        b_gate[:H].reshape(JT, P).T,
        b_gate[H:].reshape(JT, P).T,
        b_i.reshape(JT, P).T,
        b_h.reshape(JT, P).T,
    ], axis=1).astype(np.float32)
    return wi, wr, wz, wh, np.ascontiguousarray(biasp)


def kernel(input, hidden, W_gate, b_gate, W_i, b_i, W_h, b_h):
    input = np.asarray(input, dtype=np.float32)
    hidden = np.asarray(hidden, dtype=np.float32)
    W_gate = np.asarray(W_gate, dtype=np.float32)
    b_gate = np.asarray(b_gate, dtype=np.float32)
    W_i = np.asarray(W_i, dtype=np.float32)
    b_i = np.asarray(b_i, dtype=np.float32)
    W_h = np.asarray(W_h, dtype=np.float32)
    b_h = np.asarray(b_h, dtype=np.float32)

    nc = build_gru_bass()
    wi, wr, wz, wh, biasp = _pack_weights(W_gate, b_gate, W_i, b_i, W_h, b_h)

    bf16 = ml_dtypes.bfloat16
    fp8 = ml_dtypes.float8_e4m3

    def pack8(aT):  # [512, BL] fp32 -> [P, 4, BL] fp8 (scaled)
        a = np.clip(aT * ASCALE, -240.0, 240.0)
        a = a.reshape(4, P, BL).transpose(1, 0, 2)
        return np.ascontiguousarray(a.astype(fp8))

    in_maps = []
    for c in range(NCORES):
        sl = slice(c * BL, (c + 1) * BL)
        xT = np.ascontiguousarray(input[sl].T)
        hT = np.ascontiguousarray(hidden[sl].T)
        in_maps.append({
            "xb": np.ascontiguousarray(xT.astype(bf16)),
            "hb": np.ascontiguousarray(hT.astype(bf16)),
            "x8": pack8(xT),
            "h8": pack8(hT),
            "wi": wi, "wr": wr, "wz": wz, "wh": wh,
            "bias": biasp,
        })

    res = run_bass_kernel_spmd(
        nc, in_maps, list(range(NCORES)),
        trace=bool(int(os.environ.get("GRU_TRACE", "0"))),
    )
    out = np.empty((B, H), dtype=np.float32)
    for c in range(NCORES):
        out[c * BL:(c + 1) * BL, :] = res.results[c]["outT"].astype(np.float32).T
    if res.exec_time_ns is not None:
        kernel.last_exec_time_ns = res.exec_time_ns
        kernel.last_results = res
    return out


kernel.last_exec_time_ns = None
kernel.last_results = None


# revision 9
# speedup vs baseline: 1.6170x; 1.0040x over previous
"""GRUCell fused kernel for Trainium2, data-parallel over 8 NeuronCores.

Strategy (v3, mixed precision):
  - Shard batch (16384) across 8 cores -> 2048 rows/core; replicate weights.
  - r/z/h gates run as fp8e4 DoubleRow matmuls: 2 K-rows per PE cell per
    cycle = 2x tensor throughput.  Acts scaled x16, weights x512 (TRN e4m3
    max 240); the 1/8192 unscale is folded into the PSUM-reading ACT op.
    i gate + epilogue in bf16.  Whole-scheme numerics verified at 1.24e-2
    rel Fro error vs the 2e-2 budget (hardware matched the numpy model to
    4 digits on the previous variant).
  - All activations resident in SBUF so the PE stream never waits on acts;
    per output j-tile the phases run r -> z -> hg -> ig with PSUM bank
    ping-pong (r/hg on banks A0-3, z/ig on B0-3).
  - Batch-tile-outer matmul loops: each bank's accumulation completes at
    1/4 of the phase, so the ACT/DVE epilogue pipelines into the MM stream
    and the post-stream tail is one tile's chain (~3us).
  - Epilogue per tile: r=sig(r_ps/8192+br), z=sig(z_ps/8192+bz) [ACT],
    hgm=(hg_ps/8192+bh) [ACT Identity], m=hgm*r, s=(ig_ps+bi)+m [DVE],
    n=tanh(s) [ACT], d=h-n, e=z*d, o=n+e [DVE, all bf16 = 2x rate].
  - bf16 output (host upcasts); in-DMA ~7.8MB, out 2MB.
"""

import os
import numpy as np
import ml_dtypes
from contextlib import ExitStack

import concourse.bass as bass
import concourse.tile as tile
from concourse import bacc, mybir
from concourse.bass_utils import run_bass_kernel_spmd

B, I, H = 16384, 512, 512
NCORES = 8
BL = B // NCORES          # 2048 rows per core
NB = 512                  # batch tile (matmul moving free dim)
NBT = BL // NB            # 4 batch tiles per core
P = 128                   # partitions
KT = I // P               # 4 k-tiles (128) per of x/h
KS = (I + H) // P         # 8 k-subtiles across the r/z contraction
JT = H // P               # 4 output j-tiles per gate

ASCALE = 16.0             # fp8 activation scale
WSCALE = 512.0            # fp8 weight scale
INV_SCALE = 1.0 / (ASCALE * WSCALE)

FP32 = mybir.dt.float32
BF16 = mybir.dt.bfloat16
FP8 = mybir.dt.float8e4

_cache = {}


def build_gru_bass():
    """Build (once) the SPMD Bass program for one core's shard."""
    if "nc" in _cache:
        return _cache["nc"]

    nc = bacc.Bacc(
        "TRN2",
        target_bir_lowering=False,
        debug=False,
        enable_asserts=False,
        num_devices=NCORES,
    )

    # feature-major activations: bf16 x for the i gate, bf16 h for the
    # epilogue interpolation, DoubleRow-packed scaled fp8 x/h for r/z/hg.
    xb = nc.dram_tensor("xb", [I, BL], BF16, kind="ExternalInput").ap()
    hb = nc.dram_tensor("hb", [H, BL], BF16, kind="ExternalInput").ap()
    x8 = nc.dram_tensor("x8", [P, 4, BL], FP8, kind="ExternalInput").ap()
    h8 = nc.dram_tensor("h8", [P, 4, BL], FP8, kind="ExternalInput").ap()
    # bf16 weights for the i gate
    wi = nc.dram_tensor("wi", [JT, P, I], BF16, kind="ExternalInput").ap()
    # fp8 DoubleRow weights: [jt, p, ks, m] = W.T[ks*128+p, jt*128+m]*512
    wr = nc.dram_tensor("wr", [JT, P, KS, P], FP8, kind="ExternalInput").ap()
    wz = nc.dram_tensor("wz", [JT, P, KS, P], FP8, kind="ExternalInput").ap()
    wh = nc.dram_tensor("wh", [JT, P, KT, P], FP8, kind="ExternalInput").ap()
    # bias columns: 0..3 b_r per j-tile, 4..7 b_z, 8..11 b_i, 12..15 b_h
    bias = nc.dram_tensor("bias", [P, 16], FP32, kind="ExternalInput").ap()
    outT = nc.dram_tensor("outT", [H, BL], BF16, kind="ExternalOutput").ap()

    ADD = mybir.AluOpType.add
    MULT = mybir.AluOpType.mult
    SUB = mybir.AluOpType.subtract
    SIG = mybir.ActivationFunctionType.Sigmoid
    TANH = mybir.ActivationFunctionType.Tanh
    IDENT = mybir.ActivationFunctionType.Identity
    DR = mybir.MatmulPerfMode.DoubleRow

    with tile.TileContext(nc) as tc, ExitStack() as ctx:
        wpool = ctx.enter_context(tc.tile_pool(name="weights", bufs=1))
        apool = ctx.enter_context(tc.tile_pool(name="acts", bufs=1))
        ppool = ctx.enter_context(tc.tile_pool(name="psum", bufs=1, space="PSUM"))
        epool = ctx.enter_context(tc.tile_pool(name="epi", bufs=2))

        bias_s = wpool.tile([P, 16], FP32, tag="bias", name="bias_s")
        nc.sync.dma_start(out=bias_s[:], in_=bias[:, :])

        wr_s = [None] * JT
        wz_s = [None] * JT
        wh_s = [None] * JT
        wi_s = [None] * JT

        def load_w(which, jt):
            if which == "r":
                wr_s[jt] = wpool.tile([P, KS, P], FP8, tag=f"wr{jt}",
                                      name=f"wr{jt}")
                nc.sync.dma_start(out=wr_s[jt][:], in_=wr[jt, :, :, :])
            elif which == "z":
                wz_s[jt] = wpool.tile([P, KS, P], FP8, tag=f"wz{jt}",
                                      name=f"wz{jt}")
                nc.sync.dma_start(out=wz_s[jt][:], in_=wz[jt, :, :, :])
            elif which == "h":
                wh_s[jt] = wpool.tile([P, KT, P], FP8, tag=f"wh{jt}",
                                      name=f"wh{jt}")
                nc.sync.dma_start(out=wh_s[jt][:], in_=wh[jt, :, :, :])
            else:
                wi_s[jt] = wpool.tile([P, I], BF16, tag=f"wi{jt}", name=f"wi{jt}")
                nc.sync.dma_start(out=wi_s[jt][:], in_=wi[jt, :, :])

        # ---- input DMAs, in first-use order ----
        x8_s = apool.tile([P, 4, BL], FP8, tag="x8", name="x8_s")
        h8_s = apool.tile([P, 4, BL], FP8, tag="h8", name="h8_s")
        load_w("r", 0)
        nc.sync.dma_start(out=x8_s[:, 0:2, :], in_=x8[:, 0:2, :])
        nc.sync.dma_start(out=x8_s[:, 2:4, :], in_=x8[:, 2:4, :])
        nc.sync.dma_start(out=h8_s[:, 0:2, :], in_=h8[:, 0:2, :])
        nc.sync.dma_start(out=h8_s[:, 2:4, :], in_=h8[:, 2:4, :])
        load_w("h", 0)
        load_w("i", 0)
        xb_s = [None] * KT
        for kt in range(KT):
            xb_s[kt] = apool.tile([P, BL], BF16, tag=f"xb{kt}", name=f"xb{kt}")
            nc.sync.dma_start(out=xb_s[kt][:], in_=xb[kt * P:(kt + 1) * P, :])
        load_w("z", 0)
        hb_s = [None] * KT
        def load_hb(kt):
            hb_s[kt] = apool.tile([P, BL], BF16, tag=f"hb{kt}", name=f"hb{kt}")
            nc.sync.dma_start(out=hb_s[kt][:], in_=hb[kt * P:(kt + 1) * P, :])
        load_hb(0)
        for jt in range(1, JT):
            for which in ("r", "h", "i", "z"):
                load_w(which, jt)
            load_hb(jt)

        # PE warmup: ~12 matmuls on a zeroed tile, no DMA dependency, so
        # the HAM clock-gate releases to 2.4 GHz before real data arrives.
        warm = apool.tile([P, NB], BF16, tag="warm", name="warm")
        nc.vector.memset(warm[:], 0.0)
        warm_ps = ppool.tile([P, NB], FP32, tag="psA0", name="warm_ps")
        for _ in range(12):
            nc.tensor.matmul(out=warm_ps[:], lhsT=warm[:, 0:P], rhs=warm[:],
                             start=True, stop=True)

        # r/z DoubleRow chunks in DMA-arrival order: x first, then h.
        # chunk -> (acts tile, acts ks, weight ks)
        RZ_CHUNKS = [(x8_s, 0, 0), (x8_s, 2, 2), (h8_s, 0, 4), (h8_s, 2, 6)]
        HG_CHUNKS = [(h8_s, 0, 0), (h8_s, 2, 2)]

        def dr_phase(ps, w_t, chunks, bt_outer):
            nck = len(chunks)
            if bt_outer:
                for bt in range(NBT):
                    for kc in range(nck):
                        act, ks, ws = chunks[kc]
                        nc.tensor.matmul(
                            out=ps[bt][:], lhsT=w_t[:, ws:ws + 2, :],
                            rhs=act[:, ks:ks + 2, bass.ts(bt, NB)],
                            start=(kc == 0), stop=(kc == nck - 1),
                            perf_mode=DR)
            else:
                for kc in range(nck):
                    act, ks, ws = chunks[kc]
                    for bt in range(NBT):
                        nc.tensor.matmul(
                            out=ps[bt][:], lhsT=w_t[:, ws:ws + 2, :],
                            rhs=act[:, ks:ks + 2, bass.ts(bt, NB)],
                            start=(kc == 0), stop=(kc == nck - 1),
                            perf_mode=DR)

        def bf_phase(ps, w_t, acts, bt_outer):
            if bt_outer:
                for bt in range(NBT):
                    for kt in range(KT):
                        nc.tensor.matmul(
                            out=ps[bt][:], lhsT=w_t[:, kt * P:(kt + 1) * P],
                            rhs=acts[kt][:, bass.ts(bt, NB)],
                            start=(kt == 0), stop=(kt == KT - 1))
            else:
                for kt in range(KT):
                    for bt in range(NBT):
                        nc.tensor.matmul(
                            out=ps[bt][:], lhsT=w_t[:, kt * P:(kt + 1) * P],
                            rhs=acts[kt][:, bass.ts(bt, NB)],
                            start=(kt == 0), stop=(kt == KT - 1))

        # ---- main loop over output j-tiles ----
        for jt in range(JT):
            j0 = jt * P
            first = jt == 0

            # phase r: fp8 DR, K = I+H (banks A)
            r_ps = [ppool.tile([P, NB], FP32, tag=f"psA{bt}",
                               name=f"r_ps_{jt}_{bt}") for bt in range(NBT)]
            dr_phase(r_ps, wr_s[jt], RZ_CHUNKS, bt_outer=not first)
            r_s = [None] * NBT
            for bt in range(NBT):
                r_s[bt] = epool.tile([P, NB], BF16, tag=f"r_s{bt}",
                                     name=f"r_s_{jt}_{bt}")
                nc.scalar.activation(out=r_s[bt][:], in_=r_ps[bt][:], func=SIG,
                                     bias=bias_s[:, jt:jt + 1], scale=INV_SCALE)

            # phase hg: fp8 DR, K = H (banks A, freed per-bank by the r
            # sigmoids)
            hg_ps = [ppool.tile([P, NB], FP32, tag=f"psA{bt}",
                                name=f"hg_ps_{jt}_{bt}") for bt in range(NBT)]
            dr_phase(hg_ps, wh_s[jt], HG_CHUNKS, bt_outer=True)
            # hgm = hg/8192 + b_h (ACT move, frees banks A); m = hgm * r
            m = [None] * NBT
            for bt in range(NBT):
                hgm = epool.tile([P, NB], BF16, tag=f"hgm{bt}",
                                 name=f"hgm_{jt}_{bt}")
                nc.scalar.activation(out=hgm[:], in_=hg_ps[bt][:], func=IDENT,
                                     bias=bias_s[:, 12 + jt:13 + jt],
                                     scale=INV_SCALE)
                m[bt] = epool.tile([P, NB], BF16, tag=f"m{bt}",
                                   name=f"m_{jt}_{bt}")
                nc.vector.tensor_tensor(out=m[bt][:], in0=hgm[:],
                                        in1=r_s[bt][:], op=MULT)

            # phase ig: bf16, K = I (banks B)
            ig_ps = [ppool.tile([P, NB], FP32, tag=f"psB{bt}",
                                name=f"ig_ps_{jt}_{bt}") for bt in range(NBT)]
            bf_phase(ig_ps, wi_s[jt], xb_s, bt_outer=not first)
            # s = (i_gate + b_i) + m; n = tanh(s); d = h - n  (frees banks B)
            n = [None] * NBT
            d = [None] * NBT
            for bt in range(NBT):
                bsl = bass.ts(bt, NB)
                s = epool.tile([P, NB], BF16, tag=f"s{bt}", name=f"s_{jt}_{bt}")
                nc.vector.scalar_tensor_tensor(
                    out=s[:], in0=ig_ps[bt][:],
                    scalar=bias_s[:, 8 + jt:9 + jt],
                    in1=m[bt][:], op0=ADD, op1=ADD)
                n[bt] = epool.tile([P, NB], BF16, tag=f"n{bt}",
                                   name=f"n_{jt}_{bt}")
                nc.scalar.activation(out=n[bt][:], in_=s[:], func=TANH)
                d[bt] = epool.tile([P, NB], BF16, tag=f"d{bt}",
                                   name=f"d_{jt}_{bt}")
                nc.vector.tensor_tensor(
                    out=d[bt][:], in0=hb_s[jt][:, bsl], in1=n[bt][:], op=SUB)

            # phase z: fp8 DR (banks A, freed per-bank by the hgm ACT ops,
            # so no PE phase transition ever waits on the DVE); its
            # epilogue is the shortest chain, so it goes last.
            z_ps = [ppool.tile([P, NB], FP32, tag=f"psA{bt}",
                               name=f"z_ps_{jt}_{bt}") for bt in range(NBT)]
            dr_phase(z_ps, wz_s[jt], RZ_CHUNKS, bt_outer=True)
            for bt in range(NBT):
                bsl = bass.ts(bt, NB)
                z_s = epool.tile([P, NB], BF16, tag=f"z_s{bt}",
                                 name=f"z_s_{jt}_{bt}")
                nc.scalar.activation(out=z_s[:], in_=z_ps[bt][:], func=SIG,
                                     bias=bias_s[:, 4 + jt:5 + jt],
                                     scale=INV_SCALE)
                e = epool.tile([P, NB], BF16, tag=f"e{bt}", name=f"e_{jt}_{bt}")
                nc.vector.tensor_tensor(
                    out=e[:], in0=z_s[:], in1=d[bt][:], op=MULT)
                o = epool.tile([P, NB], BF16, tag=f"o{bt}", name=f"o_{jt}_{bt}")
                nc.vector.tensor_tensor(
                    out=o[:], in0=n[bt][:], in1=e[:], op=ADD)
                nc.sync.dma_start(out=outT[j0:j0 + P, bsl], in_=o[:])

    nc.compile()
    _cache["nc"] = nc
    return nc


def _pack_weights(W_gate, b_gate, W_i, b_i, W_h, b_h):
    bf16 = ml_dtypes.bfloat16
    fp8 = ml_dtypes.float8_e4m3

    def pack_bf16(WT):  # [I, H] -> [JT, P, I] with [jt, p, kt*128+m]
        a = WT.reshape(KT, P, JT, P).transpose(2, 1, 0, 3).reshape(JT, P, I)
        return np.ascontiguousarray(a.astype(bf16))

    def pack_fp8(WT):   # [K, 512] -> [JT, P, K/128, P]
        ks = WT.shape[0] // P
        a = np.clip(WT * WSCALE, -240.0, 240.0)
        a = a.reshape(ks, P, JT, P).transpose(2, 1, 0, 3)
        return np.ascontiguousarray(a.astype(fp8))

    wi = pack_bf16(W_i.T)
    wr = pack_fp8(W_gate[:H].T)
    wz = pack_fp8(W_gate[H:].T)
    wh = pack_fp8(W_h.T)
    biasp = np.concatenate([
        b_gate[:H].reshape(JT, P).T,
        b_gate[H:].reshape(JT, P).T,
        b_i.reshape(JT, P).T,
        b_h.reshape(JT, P).T,
    ], axis=1).astype(np.float32)
    return wi, wr, wz, wh, np.ascontiguousarray(biasp)


def kernel(input, hidden, W_gate, b_gate, W_i, b_i, W_h, b_h):
    input = np.asarray(input, dtype=np.float32)
    hidden = np.asarray(hidden, dtype=np.float32)
    W_gate = np.asarray(W_gate, dtype=np.float32)
    b_gate = np.asarray(b_gate, dtype=np.float32)
    W_i = np.asarray(W_i, dtype=np.float32)
    b_i = np.asarray(b_i, dtype=np.float32)
    W_h = np.asarray(W_h, dtype=np.float32)
    b_h = np.asarray(b_h, dtype=np.float32)

    nc = build_gru_bass()
    wi, wr, wz, wh, biasp = _pack_weights(W_gate, b_gate, W_i, b_i, W_h, b_h)

    bf16 = ml_dtypes.bfloat16
    fp8 = ml_dtypes.float8_e4m3

    def pack8(aT):  # [512, BL] fp32 -> [P, 4, BL] fp8 (scaled)
        a = np.clip(aT * ASCALE, -240.0, 240.0)
        a = a.reshape(4, P, BL).transpose(1, 0, 2)
        return np.ascontiguousarray(a.astype(fp8))

    in_maps = []
    for c in range(NCORES):
        sl = slice(c * BL, (c + 1) * BL)
        xT = np.ascontiguousarray(input[sl].T)
        hT = np.ascontiguousarray(hidden[sl].T)
        in_maps.append({
            "xb": np.ascontiguousarray(xT.astype(bf16)),
            "hb": np.ascontiguousarray(hT.astype(bf16)),
            "x8": pack8(xT),
            "h8": pack8(hT),
            "wi": wi, "wr": wr, "wz": wz, "wh": wh,
            "bias": biasp,
        })

    res = run_bass_kernel_spmd(
        nc, in_maps, list(range(NCORES)),
        trace=bool(int(os.environ.get("GRU_TRACE", "0"))),
    )
    out = np.empty((B, H), dtype=np.float32)
    for c in range(NCORES):
        out[c * BL:(c + 1) * BL, :] = res.results[c]["outT"].astype(np.float32).T
    if res.exec_time_ns is not None:
        kernel.last_exec_time_ns = res.exec_time_ns
        kernel.last_results = res
    return out


kernel.last_exec_time_ns = None
kernel.last_results = None


# revision 10
# speedup vs baseline: 1.6631x; 1.0286x over previous
"""GRUCell fused kernel for Trainium2, data-parallel over 8 NeuronCores.

Strategy (v3, mixed precision):
  - Shard batch (16384) across 8 cores -> 2048 rows/core; replicate weights.
  - r/z/h gates run as fp8e4 DoubleRow matmuls: 2 K-rows per PE cell per
    cycle = 2x tensor throughput.  Acts scaled x16, weights x512 (TRN e4m3
    max 240); the 1/8192 unscale is folded into the PSUM-reading ACT op.
    i gate + epilogue in bf16.  Whole-scheme numerics verified at 1.24e-2
    rel Fro error vs the 2e-2 budget (hardware matched the numpy model to
    4 digits on the previous variant).
  - All activations resident in SBUF so the PE stream never waits on acts;
    per output j-tile the phases run r -> z -> hg -> ig with PSUM bank
    ping-pong (r/hg on banks A0-3, z/ig on B0-3).
  - Batch-tile-outer matmul loops: each bank's accumulation completes at
    1/4 of the phase, so the ACT/DVE epilogue pipelines into the MM stream
    and the post-stream tail is one tile's chain (~3us).
  - Epilogue per tile: r=sig(r_ps/8192+br), z=sig(z_ps/8192+bz) [ACT],
    hgm=(hg_ps/8192+bh) [ACT Identity], m=hgm*r, s=(ig_ps+bi)+m [DVE],
    n=tanh(s) [ACT], d=h-n, e=z*d, o=n+e [DVE, all bf16 = 2x rate].
  - bf16 output (host upcasts); in-DMA ~7.8MB, out 2MB.
"""

import os
import numpy as np
import ml_dtypes
from contextlib import ExitStack

import concourse.bass as bass
import concourse.tile as tile
from concourse import bacc, mybir
from concourse.bass_utils import run_bass_kernel_spmd

B, I, H = 16384, 512, 512
NCORES = 8
BL = B // NCORES          # 2048 rows per core
NB = 512                  # batch tile (matmul moving free dim)
NBT = BL // NB            # 4 batch tiles per core
P = 128                   # partitions
KT = I // P               # 4 k-tiles (128) per of x/h
KS = (I + H) // P         # 8 k-subtiles across the r/z contraction
JT = H // P               # 4 output j-tiles per gate

ASCALE = 16.0             # fp8 activation scale
WSCALE = 512.0            # fp8 weight scale
INV_SCALE = 1.0 / (ASCALE * WSCALE)

FP32 = mybir.dt.float32
BF16 = mybir.dt.bfloat16
FP8 = mybir.dt.float8e4

_cache = {}


def build_gru_bass():
    """Build (once) the SPMD Bass program for one core's shard."""
    if "nc" in _cache:
        return _cache["nc"]

    nc = bacc.Bacc(
        "TRN2",
        target_bir_lowering=False,
        debug=False,
        enable_asserts=False,
        num_devices=NCORES,
    )

    # feature-major activations: bf16 x for the i gate, bf16 h for the
    # epilogue interpolation, DoubleRow-packed scaled fp8 x/h for r/z/hg.
    xb = nc.dram_tensor("xb", [I, BL], BF16, kind="ExternalInput").ap()
    hb = nc.dram_tensor("hb", [H, BL], BF16, kind="ExternalInput").ap()
    x8 = nc.dram_tensor("x8", [P, 4, BL], FP8, kind="ExternalInput").ap()
    h8 = nc.dram_tensor("h8", [P, 4, BL], FP8, kind="ExternalInput").ap()
    # bf16 weights for the i gate
    wi = nc.dram_tensor("wi", [JT, P, I], BF16, kind="ExternalInput").ap()
    # fp8 DoubleRow weights: [jt, p, ks, m] = W.T[ks*128+p, jt*128+m]*512
    wr = nc.dram_tensor("wr", [JT, P, KS, P], FP8, kind="ExternalInput").ap()
    wz = nc.dram_tensor("wz", [JT, P, KS, P], FP8, kind="ExternalInput").ap()
    wh = nc.dram_tensor("wh", [JT, P, KT, P], FP8, kind="ExternalInput").ap()
    # bias columns: 0..3 b_r per j-tile, 4..7 b_z, 8..11 b_i, 12..15 b_h
    bias = nc.dram_tensor("bias", [P, 16], FP32, kind="ExternalInput").ap()
    outT = nc.dram_tensor("outT", [H, BL], BF16, kind="ExternalOutput").ap()

    ADD = mybir.AluOpType.add
    MULT = mybir.AluOpType.mult
    SUB = mybir.AluOpType.subtract
    SIG = mybir.ActivationFunctionType.Sigmoid
    TANH = mybir.ActivationFunctionType.Tanh
    IDENT = mybir.ActivationFunctionType.Identity
    DR = mybir.MatmulPerfMode.DoubleRow

    with tile.TileContext(nc) as tc, ExitStack() as ctx:
        wpool = ctx.enter_context(tc.tile_pool(name="weights", bufs=1))
        apool = ctx.enter_context(tc.tile_pool(name="acts", bufs=1))
        ppool = ctx.enter_context(tc.tile_pool(name="psum", bufs=1, space="PSUM"))
        epool = ctx.enter_context(tc.tile_pool(name="epi", bufs=2))

        # PE warmup: ~12 matmuls on a zeroed tile, no DMA dependency, so
        # the HAM clock-gate releases to 2.4 GHz before real data arrives.
        # Bank psB0 is first reused by ig-jt0, well after the warmup ends.
        warm = apool.tile([P, NB], BF16, tag="warm", name="warm")
        nc.gpsimd.memset(warm[:], 0.0)
        warm_ps = ppool.tile([P, NB], FP32, tag="psB0", name="warm_ps")
        for _ in range(12):
            nc.tensor.matmul(out=warm_ps[:], lhsT=warm[:, 0:P], rhs=warm[:],
                             start=True, stop=True)

        bias_s = wpool.tile([P, 16], FP32, tag="bias", name="bias_s")
        nc.sync.dma_start(out=bias_s[:], in_=bias[:, :])

        wr_s = [None] * JT
        wz_s = [None] * JT
        wh_s = [None] * JT
        wi_s = [None] * JT

        def load_w(which, jt):
            if which == "r":
                wr_s[jt] = wpool.tile([P, KS, P], FP8, tag=f"wr{jt}",
                                      name=f"wr{jt}")
                nc.sync.dma_start(out=wr_s[jt][:], in_=wr[jt, :, :, :])
            elif which == "z":
                wz_s[jt] = wpool.tile([P, KS, P], FP8, tag=f"wz{jt}",
                                      name=f"wz{jt}")
                nc.sync.dma_start(out=wz_s[jt][:], in_=wz[jt, :, :, :])
            elif which == "h":
                wh_s[jt] = wpool.tile([P, KT, P], FP8, tag=f"wh{jt}",
                                      name=f"wh{jt}")
                nc.sync.dma_start(out=wh_s[jt][:], in_=wh[jt, :, :, :])
            else:
                wi_s[jt] = wpool.tile([P, I], BF16, tag=f"wi{jt}", name=f"wi{jt}")
                nc.sync.dma_start(out=wi_s[jt][:], in_=wi[jt, :, :])

        # ---- input DMAs, in first-use order ----
        x8_s = apool.tile([P, 4, BL], FP8, tag="x8", name="x8_s")
        h8_s = apool.tile([P, 4, BL], FP8, tag="h8", name="h8_s")
        load_w("r", 0)
        nc.sync.dma_start(out=x8_s[:, 0:2, :], in_=x8[:, 0:2, :])
        nc.sync.dma_start(out=x8_s[:, 2:4, :], in_=x8[:, 2:4, :])
        nc.sync.dma_start(out=h8_s[:, 0:2, :], in_=h8[:, 0:2, :])
        nc.sync.dma_start(out=h8_s[:, 2:4, :], in_=h8[:, 2:4, :])
        load_w("h", 0)
        load_w("i", 0)
        xb_s = [None] * KT
        for kt in range(KT):
            xb_s[kt] = apool.tile([P, BL], BF16, tag=f"xb{kt}", name=f"xb{kt}")
            nc.sync.dma_start(out=xb_s[kt][:], in_=xb[kt * P:(kt + 1) * P, :])
        load_w("z", 0)
        hb_s = [None] * KT
        def load_hb(kt):
            hb_s[kt] = apool.tile([P, BL], BF16, tag=f"hb{kt}", name=f"hb{kt}")
            nc.sync.dma_start(out=hb_s[kt][:], in_=hb[kt * P:(kt + 1) * P, :])
        load_hb(0)
        for jt in range(1, JT):
            for which in ("r", "h", "i", "z"):
                load_w(which, jt)
            load_hb(jt)

        # r/z DoubleRow chunks in DMA-arrival order: x first, then h.
        # chunk -> (acts tile, acts ks, weight ks)
        RZ_CHUNKS = [(x8_s, 0, 0), (x8_s, 2, 2), (h8_s, 0, 4), (h8_s, 2, 6)]
        HG_CHUNKS = [(h8_s, 0, 0), (h8_s, 2, 2)]

        def dr_phase(ps, w_t, chunks, bt_outer):
            nck = len(chunks)
            if bt_outer:
                for bt in range(NBT):
                    for kc in range(nck):
                        act, ks, ws = chunks[kc]
                        nc.tensor.matmul(
                            out=ps[bt][:], lhsT=w_t[:, ws:ws + 2, :],
                            rhs=act[:, ks:ks + 2, bass.ts(bt, NB)],
                            start=(kc == 0), stop=(kc == nck - 1),
                            perf_mode=DR)
            else:
                for kc in range(nck):
                    act, ks, ws = chunks[kc]
                    for bt in range(NBT):
                        nc.tensor.matmul(
                            out=ps[bt][:], lhsT=w_t[:, ws:ws + 2, :],
                            rhs=act[:, ks:ks + 2, bass.ts(bt, NB)],
                            start=(kc == 0), stop=(kc == nck - 1),
                            perf_mode=DR)

        def bf_phase(ps, w_t, acts, bt_outer):
            if bt_outer:
                for bt in range(NBT):
                    for kt in range(KT):
                        nc.tensor.matmul(
                            out=ps[bt][:], lhsT=w_t[:, kt * P:(kt + 1) * P],
                            rhs=acts[kt][:, bass.ts(bt, NB)],
                            start=(kt == 0), stop=(kt == KT - 1))
            else:
                for kt in range(KT):
                    for bt in range(NBT):
                        nc.tensor.matmul(
                            out=ps[bt][:], lhsT=w_t[:, kt * P:(kt + 1) * P],
                            rhs=acts[kt][:, bass.ts(bt, NB)],
                            start=(kt == 0), stop=(kt == KT - 1))

        # ---- main loop over output j-tiles ----
        for jt in range(JT):
            j0 = jt * P
            first = jt == 0

            # phase r: fp8 DR, K = I+H (banks A)
            r_ps = [ppool.tile([P, NB], FP32, tag=f"psA{bt}",
                               name=f"r_ps_{jt}_{bt}") for bt in range(NBT)]
            dr_phase(r_ps, wr_s[jt], RZ_CHUNKS, bt_outer=not first)
            r_s = [None] * NBT
            for bt in range(NBT):
                r_s[bt] = epool.tile([P, NB], BF16, tag=f"r_s{bt}",
                                     name=f"r_s_{jt}_{bt}")
                nc.scalar.activation(out=r_s[bt][:], in_=r_ps[bt][:], func=SIG,
                                     bias=bias_s[:, jt:jt + 1], scale=INV_SCALE)

            # phase hg: fp8 DR, K = H (banks A, freed per-bank by the r
            # sigmoids)
            hg_ps = [ppool.tile([P, NB], FP32, tag=f"psA{bt}",
                                name=f"hg_ps_{jt}_{bt}") for bt in range(NBT)]
            dr_phase(hg_ps, wh_s[jt], HG_CHUNKS, bt_outer=True)
            # hgm = hg/8192 + b_h (ACT move, frees banks A); m = hgm * r
            m = [None] * NBT
            for bt in range(NBT):
                hgm = epool.tile([P, NB], BF16, tag=f"hgm{bt}",
                                 name=f"hgm_{jt}_{bt}")
                nc.scalar.activation(out=hgm[:], in_=hg_ps[bt][:], func=IDENT,
                                     bias=bias_s[:, 12 + jt:13 + jt],
                                     scale=INV_SCALE)
                m[bt] = epool.tile([P, NB], BF16, tag=f"m{bt}",
                                   name=f"m_{jt}_{bt}")
                nc.vector.tensor_tensor(out=m[bt][:], in0=hgm[:],
                                        in1=r_s[bt][:], op=MULT)

            # phases ig (banks B) and z (banks A, freed per-bank by hgm):
            # interleaved per batch-tile for jt>=1 so each tile's epilogue
            # drains during the remaining matmuls and the post-stream tail
            # is one tile's z_s -> e -> o chain.
            ig_ps = [ppool.tile([P, NB], FP32, tag=f"psB{bt}",
                                name=f"ig_ps_{jt}_{bt}") for bt in range(NBT)]
            z_ps = [ppool.tile([P, NB], FP32, tag=f"psA{bt}",
                               name=f"z_ps_{jt}_{bt}") for bt in range(NBT)]

            def ig_mms(bt):
                for kt in range(KT):
                    nc.tensor.matmul(
                        out=ig_ps[bt][:], lhsT=wi_s[jt][:, kt * P:(kt + 1) * P],
                        rhs=xb_s[kt][:, bass.ts(bt, NB)],
                        start=(kt == 0), stop=(kt == KT - 1))

            def z_mms(bt):
                for kc in range(len(RZ_CHUNKS)):
                    act, ks, ws = RZ_CHUNKS[kc]
                    nc.tensor.matmul(
                        out=z_ps[bt][:], lhsT=wz_s[jt][:, ws:ws + 2, :],
                        rhs=act[:, ks:ks + 2, bass.ts(bt, NB)],
                        start=(kc == 0), stop=(kc == len(RZ_CHUNKS) - 1),
                        perf_mode=DR)

            def ig_epi(bt):
                # s = (i_gate + b_i) + m; n = tanh(s); d = h - n
                bsl = bass.ts(bt, NB)
                s = epool.tile([P, NB], BF16, tag=f"s{bt}", name=f"s_{jt}_{bt}")
                nc.vector.scalar_tensor_tensor(
                    out=s[:], in0=ig_ps[bt][:],
                    scalar=bias_s[:, 8 + jt:9 + jt],
                    in1=m[bt][:], op0=ADD, op1=ADD)
                n[bt] = epool.tile([P, NB], BF16, tag=f"n{bt}",
                                   name=f"n_{jt}_{bt}")
                nc.scalar.activation(out=n[bt][:], in_=s[:], func=TANH)
                d[bt] = epool.tile([P, NB], BF16, tag=f"d{bt}",
                                   name=f"d_{jt}_{bt}")
                nc.vector.tensor_tensor(
                    out=d[bt][:], in0=hb_s[jt][:, bsl], in1=n[bt][:], op=SUB)

            def z_epi(bt):
                # z = sig(z_ps/8192 + b_z); h' = n + z*d
                bsl = bass.ts(bt, NB)
                z_s = epool.tile([P, NB], BF16, tag=f"z_s{bt}",
                                 name=f"z_s_{jt}_{bt}")
                nc.scalar.activation(out=z_s[:], in_=z_ps[bt][:], func=SIG,
                                     bias=bias_s[:, 4 + jt:5 + jt],
                                     scale=INV_SCALE)
                e = epool.tile([P, NB], BF16, tag=f"e{bt}", name=f"e_{jt}_{bt}")
                nc.vector.tensor_tensor(
                    out=e[:], in0=z_s[:], in1=d[bt][:], op=MULT)
                o = epool.tile([P, NB], BF16, tag=f"o{bt}", name=f"o_{jt}_{bt}")
                nc.vector.tensor_tensor(
                    out=o[:], in0=n[bt][:], in1=e[:], op=ADD)
                nc.sync.dma_start(out=outT[j0:j0 + P, bsl], in_=o[:])

            n = [None] * NBT
            d = [None] * NBT
            if first:
                # DMA-arrival-tolerant: ig k-tile-outer, then z, epilogues after
                bf_phase(ig_ps, wi_s[jt], xb_s, bt_outer=False)
                for bt in range(NBT):
                    ig_epi(bt)
                dr_phase(z_ps, wz_s[jt], RZ_CHUNKS, bt_outer=True)
                for bt in range(NBT):
                    z_epi(bt)
            else:
                for bt in range(NBT):
                    ig_mms(bt)
                    ig_epi(bt)
                    z_mms(bt)
                    z_epi(bt)

    nc.compile()
    _cache["nc"] = nc
    return nc


def _pack_weights(W_gate, b_gate, W_i, b_i, W_h, b_h):
    bf16 = ml_dtypes.bfloat16
    fp8 = ml_dtypes.float8_e4m3

    def pack_bf16(WT):  # [I, H] -> [JT, P, I] with [jt, p, kt*128+m]
        a = WT.reshape(KT, P, JT, P).transpose(2, 1, 0, 3).reshape(JT, P, I)
        return np.ascontiguousarray(a.astype(bf16))

    def pack_fp8(WT):   # [K, 512] -> [JT, P, K/128, P]
        ks = WT.shape[0] // P
        a = np.clip(WT * WSCALE, -240.0, 240.0)
        a = a.reshape(ks, P, JT, P).transpose(2, 1, 0, 3)
        return np.ascontiguousarray(a.astype(fp8))

    wi = pack_bf16(W_i.T)
    wr = pack_fp8(W_gate[:H].T)
    wz = pack_fp8(W_gate[H:].T)
    wh = pack_fp8(W_h.T)
    biasp = np.concatenate([
        b_gate[:H].reshape(JT, P).T,
        b_gate[H:].reshape(JT, P).T,
        b_i.reshape(JT, P).T,
        b_h.reshape(JT, P).T,
    ], axis=1).astype(np.float32)
    return wi, wr, wz, wh, np.ascontiguousarray(biasp)


def kernel(input, hidden, W_gate, b_gate, W_i, b_i, W_h, b_h):
    input = np.asarray(input, dtype=np.float32)
    hidden = np.asarray(hidden, dtype=np.float32)
    W_gate = np.asarray(W_gate, dtype=np.float32)
    b_gate = np.asarray(b_gate, dtype=np.float32)
    W_i = np.asarray(W_i, dtype=np.float32)
    b_i = np.asarray(b_i, dtype=np.float32)
    W_h = np.asarray(W_h, dtype=np.float32)
    b_h = np.asarray(b_h, dtype=np.float32)

    nc = build_gru_bass()
    wi, wr, wz, wh, biasp = _pack_weights(W_gate, b_gate, W_i, b_i, W_h, b_h)

    bf16 = ml_dtypes.bfloat16
    fp8 = ml_dtypes.float8_e4m3

    def pack8(aT):  # [512, BL] fp32 -> [P, 4, BL] fp8 (scaled)
        a = np.clip(aT * ASCALE, -240.0, 240.0)
        a = a.reshape(4, P, BL).transpose(1, 0, 2)
        return np.ascontiguousarray(a.astype(fp8))

    in_maps = []
    for c in range(NCORES):
        sl = slice(c * BL, (c + 1) * BL)
        xT = np.ascontiguousarray(input[sl].T)
        hT = np.ascontiguousarray(hidden[sl].T)
        in_maps.append({
            "xb": np.ascontiguousarray(xT.astype(bf16)),
            "hb": np.ascontiguousarray(hT.astype(bf16)),
            "x8": pack8(xT),
            "h8": pack8(hT),
            "wi": wi, "wr": wr, "wz": wz, "wh": wh,
            "bias": biasp,
        })

    res = run_bass_kernel_spmd(
        nc, in_maps, list(range(NCORES)),
        trace=bool(int(os.environ.get("GRU_TRACE", "0"))),
    )
    out = np.empty((B, H), dtype=np.float32)
    for c in range(NCORES):
        out[c * BL:(c + 1) * BL, :] = res.results[c]["outT"].astype(np.float32).T
    if res.exec_time_ns is not None:
        kernel.last_exec_time_ns = res.exec_time_ns
        kernel.last_results = res
    return out


kernel.last_exec_time_ns = None
kernel.last_results = None
